# revision 8
# baseline (speedup 1.0000x reference)
"""Tensor-parallel causal GQA self-attention (B=1, S=2048, D=4096, 32 q heads /
8 kv heads, HD=128, interleaved RoPE) on 8 trn2 NeuronCores.

Sharding: core c owns kv head c and q heads 4c..4c+3 (column-parallel
Wq/Wk/Wv, row-parallel Wo).  Each core computes a full [S, D] partial of the
output projection; the host sums the 8 partials (the "all-reduce").

fp8 DoubleRow strategy: fp8e4 + MatmulPerfMode.DoubleRow runs at 0.5
cycles/output-column and sums TWO K=128 products per instruction (4x the
fp32r MAC rate).  Heavy-tailed outputs make single-fp8 operands too lossy
anywhere in the main signal path, so the big GEMMs use a hi+lo split
(x = fp8(x) + fp8(x - fp8(x)), ~9 significand bits) in 3 passes
(xh@Wh + xl@Wh + xh@Wl) at 0.75x the fp32r cost:
  QKV     3-pass DoubleRow over k-tile pairs (hi/lo of x and W host-prepped)
  scores  stay bf16/fp32-rate (K=128, full precision; same cost as fp32r)
  P       exp(s-2.5) via ScalarE straight from PSUM to fp8e4 (e4m3 max 240,
          max score 7.36 -> max p 130); denominator sums the same quantized
          values so the softmax reweighting error mostly cancels
  AV      DoubleRow pairs of j-tiles, V in hi/lo (2 instrs per j-pair)
  denom   ones-matrix DoubleRow over the same j-pairs (1 instr per pair)
  o_proj  3-pass DoubleRow over head pairs, otn in hi/lo (bf16 staging)
Scale folding: Wq x2^10, Wk/Wv/Wo x2^7 (host), descaled via rope tables /
the denominator ones value (2^(7-2)) / a single host-side /512 of the
summed bf16 partials.
"""

import sys

if "/opt/trn_rl_repo" not in sys.path:
    sys.path.insert(0, "/opt/trn_rl_repo")

import numpy as np
import ml_dtypes

import concourse.bass as bass
import concourse.tile as tile
from concourse import bacc, mybir
from concourse.bass_utils import run_bass_kernel_spmd

S, D, NH, NKV, HD = 2048, 4096, 32, 8, 128
NCORES = 8
QH = NH // NCORES  # 4 q heads per core
ROPE_BASE = 500000.0

F32 = mybir.dt.float32
F32R = mybir.dt.float32r
BF16 = mybir.dt.bfloat16
E4 = mybir.dt.float8e4
AF = mybir.ActivationFunctionType
DR = mybir.MatmulPerfMode.DoubleRow

NE4 = ml_dtypes.float8_e4m3
NBF = ml_dtypes.bfloat16

SC = S // 512   # 4 s-chunks of 512
DP = D // 256   # 16 k-tile pairs
JT = S // 128   # 16 j-tiles of 128

SWQ = 10        # Wq*scale scaled by 2^10 before fp8
SWK = 7         # Wk, Wv, Wo scaled by 2^7
SWV = 7         # Wv host scale; V descaled by 2^(SVV-SWV) at the PSUM copy
SVV = 2         # V's effective fp8 scale (max |4v| ~ 19 < 240)
SWO = 7
SO = 2          # otn scaled by 2^2
CEXP = 2.5      # exp(s - CEXP); max causal score ~7.36 -> max p ~130 < 240

_CACHE = {}

TRACE = False
LAST_EXEC_NS = None


def _pair2(ap):
    """[128, 2*n] -> [128, 2, n] DoubleRow view."""
    n = ap.shape[-1] // 2
    return ap.rearrange("p (two n) -> p two n", two=2, n=n)


def _build_nc():
    nc = bacc.Bacc("TRN2", target_bir_lowering=False, debug=False,
                   num_devices=NCORES)

    xw_d = nc.declare_dram_parameter("xw", [SC, DP, 128, 3072], E4,
                                     isOutput=False)
    wq_d = nc.declare_dram_parameter("wq2", [DP, 128, 2048], E4,
                                     isOutput=False)
    wo_d = nc.declare_dram_parameter("wo4", [8, 128, 4096], E4,
                                     isOutput=False)
    cs_d = nc.declare_dram_parameter("cs4", [SC, HD, 2048], BF16,
                                     isOutput=False)
    perm_d = nc.declare_dram_parameter("perm", [HD, HD], BF16, isOutput=False)
    masks_d = nc.declare_dram_parameter("masks", [128, 4, 512], BF16,
                                        isOutput=False)
    onr_d = nc.declare_dram_parameter("ones_red", [128, 256], E4,
                                      isOutput=False)
    onb_d = nc.declare_dram_parameter("ones_bc", [1, 128], F32R,
                                      isOutput=False)
    ident_d = nc.declare_dram_parameter("ident", [HD, HD], BF16,
                                        isOutput=False)
    out_d = nc.declare_dram_parameter("out", [8, 8, 128, 1024], BF16,
                                      isOutput=True)

    with tile.TileContext(nc) as tc:
        from contextlib import ExitStack
        ctx = ExitStack()
        with ctx:
            wpool = ctx.enter_context(tc.tile_pool(name="wpool", bufs=16))
            xpool = ctx.enter_context(tc.tile_pool(name="xpool", bufs=3))
            qtp = ctx.enter_context(tc.tile_pool(name="qtp", bufs=9))
            otnp = ctx.enter_context(tc.tile_pool(name="otnp", bufs=16))
            tabp = ctx.enter_context(tc.tile_pool(name="tabp", bufs=2))
            ktp = ctx.enter_context(tc.tile_pool(name="ktp", bufs=4))
            vnp = ctx.enter_context(tc.tile_pool(name="vnp", bufs=8))
            stg = ctx.enter_context(tc.tile_pool(name="stg", bufs=4))
            rawp = ctx.enter_context(tc.tile_pool(name="rawp", bufs=6))
            ptp = ctx.enter_context(tc.tile_pool(name="ptp", bufs=4))
            mkp = ctx.enter_context(tc.tile_pool(name="mkp", bufs=1))
            cst = ctx.enter_context(tc.tile_pool(name="cst", bufs=1))
            rcp = ctx.enter_context(tc.tile_pool(name="rcp", bufs=4))
            ostp = ctx.enter_context(tc.tile_pool(name="ostp", bufs=3))
            psA = ctx.enter_context(
                tc.tile_pool(name="psA", bufs=6, space=bass.MemorySpace.PSUM))
            psB = ctx.enter_context(
                tc.tile_pool(name="psB", bufs=2, space=bass.MemorySpace.PSUM))

            perm_t = cst.tile([HD, HD], BF16, name="perm_t")
            ident_t = cst.tile([HD, HD], BF16, name="ident_t")
            onr_t = cst.tile([128, 256], E4, name="onr_t")
            onb_t = cst.tile([1, 128], F32R, name="onb_t")
            nexp_t = cst.tile([128, 1], F32, name="nexp_t")
            mask_t = mkp.tile([128, 4, 512], BF16, name="mask4")

            def table_loads():
                yield lambda: nc.sync.dma_start(perm_t[:], perm_d[:])
                yield lambda: nc.scalar.dma_start(ident_t[:], ident_d[:])
                yield lambda: nc.sync.dma_start(onr_t[:], onr_d[:])
                yield lambda: nc.scalar.dma_start(onb_t[:], onb_d[:])
                yield lambda: nc.sync.dma_start(mask_t[:], masks_d[:])
                yield lambda: nc.gpsimd.memset(nexp_t[:], -CEXP)

            wq_tiles = [None] * DP

            # persistent activations (bf16 for q/k, fp8 hi/lo pairs for V)
            QTc = [[qtp.tile([HD, 512], BF16, tag="qtc", name=f"qt{h}_{c}")
                    for c in range(SC)] for h in range(QH)]
            KTc = [ktp.tile([HD, 512], BF16, tag="ktc", name=f"kt{c}")
                   for c in range(SC)]
            Vhc = [vnp.tile([128, 4, 128], E4, tag="vnc", name=f"vh{c}")
                   for c in range(SC)]
            Vlc = [vnp.tile([128, 4, 128], E4, tag="vnc", name=f"vl{c}")
                   for c in range(SC)]

            def rope_copy(acc_ps, eng):
                raw = rawp.tile([128, 512], BF16, tag="raw", name="rope_raw")
                if eng == "act":
                    nc.scalar.activation(raw[:], acc_ps[:], AF.Copy)
                else:
                    nc.vector.tensor_copy(raw[:], acc_ps[:])
                return raw

            def rope_combine(raw, dest, cc, sn):
                rot = psB.tile([128, 512], F32, tag="tmp", name="rope_rot")
                nc.tensor.matmul(rot[:], perm_t[:], raw[:], start=True,
                                 stop=True)
                t1 = stg.tile([128, 512], BF16, tag="stg", name="rope_t1")
                nc.vector.tensor_mul(t1[:], raw[:], cc)
                t2 = stg.tile([128, 512], BF16, tag="stg", name="rope_t2")
                nc.vector.tensor_mul(t2[:], rot[:], sn)
                nc.vector.tensor_add(dest[:], t1[:], t2[:])

            boundary_pe = [None]

            def emit_boundary_pe():
                if boundary_pe[0] is not None:
                    boundary_pe[0]()
                    boundary_pe[0] = None

            cs_tiles = [None] * SC

            # otn hi/lo head-pair tiles: [c][hp] slot h%2
            otn_hi = [[None, None] for _ in range(SC)]
            otn_lo = [[None, None] for _ in range(SC)]
            tails = []

            def make_tail(c, h, ot, dsum):
                def tail():
                    dsg = rcp.tile([1, 512], F32R, tag="rc", name="dsg")
                    rc = rcp.tile([1, 512], F32R, tag="rc", name="rc")
                    with nc.allow_low_precision(reason="fp22 softmax recip"):
                        nc.vector.tensor_scalar_max(dsg[:], dsum[0:1, :],
                                                    1e-30)
                        nc.vector.reciprocal(rc[:], dsg[:])
                    bc = psA.tile([128, 512], F32, tag="acc", name="bc")
                    nc.tensor.matmul(bc[:], onb_t[:], rc[:], start=True,
                                     stop=True)
                    bcs = stg.tile([128, 512], BF16, tag="stg", name="bcs")
                    nc.scalar.activation(bcs[:], bc[:], AF.Copy)
                    ob = stg.tile([128, 512], BF16, tag="stg", name="otn_bf")
                    nc.vector.tensor_mul(ob[:], ot[:], bcs[:])
                    hi = otn_hi[c][h // 2][:, h % 2, :]
                    nc.vector.tensor_copy(hi, ob[:])
                    nc.vector.tensor_sub(otn_lo[c][h // 2][:, h % 2, :],
                                         ob[:], hi)
                return tail

            def attn_chunk(c):
                for hp in range(2):
                    otn_hi[c][hp] = otnp.tile([128, 2, 512], E4, tag="otn",
                                              name=f"oh{c}_{hp}")
                    otn_lo[c][hp] = otnp.tile([128, 2, 512], E4, tag="otn",
                                              name=f"ol{c}_{hp}")
                for h in range(QH):
                    qch = QTc[h][c][:]
                    ot = psA.tile([128, 512], F32, tag="acc", name="ot_ps")
                    dsum = psB.tile([128, 512], F32, tag="tmp", name="dsum")
                    njt = 4 * c + 4
                    npair = njt // 2
                    pairs = [None] * npair

                    def score_pair(pr, c=c, qch=qch):
                        pp = ptp.tile([128, 2, 512], E4, tag="pt", name="pt")
                        for jj in range(2):
                            jt = 2 * pr + jj
                            stp = psA.tile([128, 512], F32, tag="acc",
                                           name="stp")
                            nc.tensor.matmul(
                                stp[:],
                                KTc[jt // 4][:, 128 * (jt % 4):
                                             128 * (jt % 4 + 1)],
                                qch, start=True, stop=True)
                            pslot = pp[:, jj, :]
                            m = jt - 4 * c
                            if m >= 0:
                                # exp can exceed fp8 range above the diagonal;
                                # stage in bf16 so inf*0 never reaches pt
                                pe = stg.tile([128, 512], BF16, tag="stg",
                                              name="pe_t")
                                nc.scalar.activation(pe[:], stp[:], AF.Exp,
                                                     bias=nexp_t[:])
                                nc.vector.tensor_mul(pslot, pe[:],
                                                     mask_t[:, m, :])
                            else:
                                nc.scalar.activation(pslot, stp[:], AF.Exp,
                                                     bias=nexp_t[:])
                        return pp

                    def accum(pr, pp, ot=ot, dsum=dsum, npair=npair):
                        st = pr == 0
                        sp = pr == npair - 1
                        vh = Vhc[pr // 2][:, 2 * (pr % 2):2 * (pr % 2) + 2, :]
                        vl = Vlc[pr // 2][:, 2 * (pr % 2):2 * (pr % 2) + 2, :]
                        nc.tensor.matmul(ot[:], vh, pp[:], start=st,
                                         stop=False, perf_mode=DR)
                        nc.tensor.matmul(ot[:], vl, pp[:], start=False,
                                         stop=sp, perf_mode=DR)
                        nc.tensor.matmul(dsum[:], _pair2(onr_t[:]), pp[:],
                                         start=st, stop=sp, perf_mode=DR)

                    for pr in range(npair):
                        pairs[pr] = score_pair(pr)
                        if pr >= 2:
                            accum(pr - 2, pairs[pr - 2])
                    for k in (2, 1):
                        accum(npair - k, pairs[npair - k])
                    tails.append(make_tail(c, h, ot, dsum))
                    if len(tails) > 1:
                        tails.pop(0)()
                while tails:
                    tails.pop(0)()

            emit_attn = [attn_chunk]

            for sc in range(SC):
                qps = [psA.tile([128, 512], F32, tag="acc", name=f"qps{h}")
                       for h in range(QH)]
                kps = psA.tile([128, 512], F32, tag="acc", name="kps")
                vps = psA.tile([128, 512], F32, tag="acc", name="vps")
                for dp in range(DP):
                    if sc == 0:
                        wt = wpool.tile([128, 2048], E4, tag="w",
                                        name=f"wq{dp}")
                        nc.scalar.dma_start(wt[:], wq_d[dp])
                        wq_tiles[dp] = wt
                    xt = xpool.tile([128, 3072], E4, tag="x", name="xt")
                    nc.sync.dma_start(xt[:], xw_d[sc, dp])
                    if sc == 0:
                        if dp == 0:
                            _tl = table_loads()
                        next(_tl, lambda: None)()
                    if dp == 1:
                        emit_boundary_pe()
                    if dp == 8:
                        cc4 = tabp.tile([128, 2048], BF16, tag="tab",
                                        name="cc4")
                        nc.sync.dma_start(cc4[:], cs_d[sc])
                        cs_tiles[sc] = cc4
                    xh = _pair2(xt[:, 0:1024])
                    xl = _pair2(xt[:, 1024:2048])
                    wkh = _pair2(xt[:, 2048:2304])
                    wkl = _pair2(xt[:, 2304:2560])
                    wvh = _pair2(xt[:, 2560:2816])
                    wvl = _pair2(xt[:, 2816:3072])
                    wqh = _pair2(wq_tiles[dp][:, 0:1024])
                    wql = _pair2(wq_tiles[dp][:, 1024:2048])
                    st = dp == 0
                    sp = dp == DP - 1
                    for h in range(QH):
                        hsl = slice(HD * h, HD * (h + 1))
                        nc.tensor.matmul(qps[h][:], wqh[:, :, hsl], xh[:],
                                         start=st, stop=False, perf_mode=DR)
                        nc.tensor.matmul(qps[h][:], wqh[:, :, hsl], xl[:],
                                         start=False, stop=False,
                                         perf_mode=DR)
                        nc.tensor.matmul(qps[h][:], wql[:, :, hsl], xh[:],
                                         start=False, stop=sp, perf_mode=DR)
                    nc.tensor.matmul(kps[:], wkh, xh[:], start=st,
                                     stop=False, perf_mode=DR)
                    nc.tensor.matmul(kps[:], wkh, xl[:], start=False,
                                     stop=False, perf_mode=DR)
                    nc.tensor.matmul(kps[:], wkl, xh[:], start=False,
                                     stop=sp, perf_mode=DR)
                    nc.tensor.matmul(vps[:], wvh, xh[:], start=st,
                                     stop=False, perf_mode=DR)
                    nc.tensor.matmul(vps[:], wvh, xl[:], start=False,
                                     stop=False, perf_mode=DR)
                    nc.tensor.matmul(vps[:], wvl, xh[:], start=False,
                                     stop=sp, perf_mode=DR)

                raw_k = rope_copy(kps, "act")
                vt_sb = rawp.tile([128, 512], BF16, tag="raw", name="vt_sb")
                nc.vector.tensor_scalar_mul(vt_sb[:], vps[:],
                                            2.0 ** (SVV - SWV))
                raw_q = [None] * QH
                raw_q[0] = rope_copy(qps[0], "dve")

                def boundary(sc=sc, raw_k=raw_k, vt_sb=vt_sb, raw_q=raw_q):
                    cc4 = cs_tiles[sc]
                    cq = cc4[:, 0:512]
                    sq = cc4[:, 512:1024]
                    ck = cc4[:, 1024:1536]
                    sk = cc4[:, 1536:2048]
                    rope_combine(raw_k, KTc[sc], ck, sk)
                    rope_combine(raw_q[0], QTc[0][sc], cq, sq)
                    for k4 in range(4):
                        vtp = psB.tile([128, 128], BF16, tag="tmp",
                                       name="vtp")
                        nc.tensor.transpose(
                            vtp[:], vt_sb[:, 128 * k4:128 * (k4 + 1)],
                            ident_t[:])
                        nc.scalar.activation(Vhc[sc][:, k4, :], vtp[:],
                                             AF.Copy)
                        nc.vector.tensor_sub(Vlc[sc][:, k4, :], vtp[:],
                                             Vhc[sc][:, k4, :])
                    for h in range(1, QH):
                        rope_combine(raw_q[h], QTc[h][sc], cq, sq)

                boundary_pe[0] = boundary
                if sc >= 1:
                    emit_attn[0](sc - 1)
                for h in range(1, QH):
                    raw_q[h] = rope_copy(qps[h],
                                         "act" if h % 2 == 0 else "dve")
                if sc == SC - 1:
                    wo_tiles = []
                    for dc in range(8):
                        wo = wpool.tile([128, 4096], E4, tag="w",
                                        name=f"wo{dc}")
                        nc.sync.dma_start(wo[:], wo_d[dc])
                        wo_tiles.append(wo)

                    emit_boundary_pe()
                    emit_attn[0](SC - 1)

            # ---- phase 3: output projection (row-parallel partial) ----
            def o_proj_all():
                for dc in range(8):
                    wo = wo_tiles[dc]
                    woh = [_pair2(wo[:, 0:1024]), _pair2(wo[:, 1024:2048])]
                    wol = [_pair2(wo[:, 2048:3072]), _pair2(wo[:, 3072:4096])]
                    for c in range(SC):
                        for lp in range(2):
                            ost = ostp.tile([128, 1024], BF16, tag="ost",
                                            name="ost")
                            for k2 in range(2):
                                kk = 2 * lp + k2
                                ksl = slice(128 * kk, 128 * (kk + 1))
                                acc = psA.tile([128, 512], F32, tag="acc",
                                               name="oacc")
                                for hp in range(2):
                                    oh = otn_hi[c][hp][:, :, ksl]
                                    ol = otn_lo[c][hp][:, :, ksl]
                                    nc.tensor.matmul(
                                        acc[:], oh, woh[hp][:],
                                        start=(hp == 0), stop=False,
                                        perf_mode=DR)
                                    nc.tensor.matmul(
                                        acc[:], ol, woh[hp][:],
                                        start=False, stop=False,
                                        perf_mode=DR)
                                    nc.tensor.matmul(
                                        acc[:], oh, wol[hp][:],
                                        start=False, stop=(hp == 1),
                                        perf_mode=DR)
                                dstc = ost[:, 512 * k2:512 * (k2 + 1)]
                                if k2 == 0:
                                    nc.vector.tensor_copy(dstc, acc[:])
                                else:
                                    nc.scalar.activation(dstc, acc[:],
                                                         AF.Copy)
                            eng = nc.sync if lp == 0 else nc.scalar
                            eng.dma_start(out_d[dc, 2 * c + lp], ost[:])

            o_proj_all()

    nc.compile()
    return nc


def _q8(x):
    return x.astype(NE4)


def _split8(x):
    h = x.astype(NE4)
    l = (x - h.astype(np.float32)).astype(NE4)
    return h, l


def _host_tables():
    pos = np.arange(S, dtype=np.float64)
    inv_freq = ROPE_BASE ** (-np.arange(0, HD, 2, dtype=np.float64) / HD)
    ang = np.outer(pos, inv_freq)  # [S, HD/2]
    cos = np.cos(ang).T.astype(np.float32)  # [HD/2, S]
    sin = np.sin(ang).T.astype(np.float32)
    cos2 = np.repeat(cos, 2, axis=0)  # [HD, S]
    sin2 = np.repeat(sin, 2, axis=0)
    sin2[0::2, :] *= -1.0  # even rows get -sin, odd rows +sin

    perm = np.zeros((HD, HD), dtype=np.float32)
    for i in range(HD):
        perm[i ^ 1, i] = 1.0

    masks = np.zeros((128, 4, 512), dtype=np.float32)
    jr = np.arange(128)[:, None]
    ir = np.arange(512)[None, :]
    for m in range(4):
        masks[:, m, :] = np.where(jr + 128 * m <= ir, 1.0, 0.0)

    return cos2, sin2, perm, masks


def kernel(x, Wq, Wk, Wv, Wo):
    global LAST_EXEC_NS
    if "nc" not in _CACHE:
        _CACHE["nc"] = _build_nc()
    nc = _CACHE["nc"]

    x = np.asarray(x, dtype=np.float32).reshape(S, D)
    Wq = np.asarray(Wq, dtype=np.float32)
    Wk = np.asarray(Wk, dtype=np.float32)
    Wv = np.asarray(Wv, dtype=np.float32)
    Wo = np.asarray(Wo, dtype=np.float32)

    xT = np.ascontiguousarray(x.T)  # [D, S]
    xh8, xl8 = _split8(xT)
    # [DP, 128, 2, 512] per (pair, row, slot, s-chunk col) built per chunk
    xh8r = xh8.reshape(DP, 2, 128, SC, 512)
    xl8r = xl8.reshape(DP, 2, 128, SC, 512)
    cos2, sin2, perm, masks = _host_tables()
    scale = np.float32(1.0 / np.sqrt(HD))

    # cs4[sc] = [cq | sq | ck | sk], each [HD, 512], bf16
    cs4 = np.empty((SC, HD, 2048), dtype=NBF)
    for scc in range(SC):
        cs = slice(512 * scc, 512 * (scc + 1))
        cs4[scc, :, 0:512] = (cos2[:, cs] * 2.0 ** -SWQ).astype(NBF)
        cs4[scc, :, 512:1024] = (sin2[:, cs] * 2.0 ** -SWQ).astype(NBF)
        cs4[scc, :, 1024:1536] = (cos2[:, cs] * 2.0 ** -SWK).astype(NBF)
        cs4[scc, :, 1536:2048] = (sin2[:, cs] * 2.0 ** -SWK).astype(NBF)

    ident = np.eye(HD, dtype=NBF)
    ones_red = np.zeros((128, 256), dtype=NE4)
    ones_red[:, 0] = 2.0 ** (SVV - SO)
    ones_red[:, 128] = 2.0 ** (SVV - SO)
    ones_bc = np.ones((1, 128), dtype=np.float32)

    in_maps = []
    for c in range(NCORES):
        qs = slice(QH * HD * c, QH * HD * (c + 1))
        ks = slice(HD * c, HD * (c + 1))
        wkh, wkl = _split8(Wk[:, ks] * 2.0 ** SWK)   # [D, 128] fp8
        wvh, wvl = _split8(Wv[:, ks] * 2.0 ** SWV)

        # xw[sc, dp] = [xh pair 1024 | xl pair 1024 | wkh 256 | wkl | wvh | wvl]
        xw = np.empty((SC, DP, 128, 3072), dtype=NE4)
        for scc in range(SC):
            xw[scc, :, :, 0:1024] = (
                xh8r[:, :, :, scc].transpose(0, 2, 1, 3).reshape(DP, 128, 1024))
            xw[scc, :, :, 1024:2048] = (
                xl8r[:, :, :, scc].transpose(0, 2, 1, 3).reshape(DP, 128, 1024))
        wk4 = np.stack([wkh, wkl, wvh, wvl], axis=0)  # [4, D, 128]
        wk4r = (wk4.reshape(4, DP, 2, 128, 128).transpose(1, 3, 0, 2, 4)
                .reshape(DP, 128, 1024))
        xw[:, :, :, 2048:3072] = wk4r[None]

        wqc = Wq[:, qs].astype(np.float32) * scale * 2.0 ** SWQ
        wqhh, wqll = _split8(wqc)   # [D, 512]
        wq2 = np.empty((DP, 128, 2048), dtype=NE4)
        wq2[:, :, 0:1024] = (wqhh.reshape(DP, 2, 128, 512)
                             .transpose(0, 2, 1, 3).reshape(DP, 128, 1024))
        wq2[:, :, 1024:2048] = (wqll.reshape(DP, 2, 128, 512)
                                .transpose(0, 2, 1, 3).reshape(DP, 128, 1024))

        # wo4[dc] = [Woh hp0 | Woh hp1 | Wol hp0 | Wol hp1], each [128,2,512]
        woc = Wo[qs, :].astype(np.float32) * 2.0 ** SWO  # [512, D]
        woh, wol = _split8(woc)
        wo4 = np.empty((8, 128, 4096), dtype=NE4)
        for part, w8 in ((0, woh), (1, wol)):
            # w8 [512, 4096] -> [4 heads, 128, 8 dc, 512]
            wr = w8.reshape(4, 128, 8, 512)
            for dc in range(8):
                for hp in range(2):
                    blk = wr[2 * hp:2 * hp + 2, :, dc, :]  # [2, 128, 512]
                    off = 2048 * part + 1024 * hp
                    wo4[dc, :, off:off + 1024] = (
                        blk.transpose(1, 0, 2).reshape(128, 1024))

        in_maps.append({
            "xw": xw,
            "wq2": wq2,
            "wo4": wo4,
            "cs4": cs4,
            "perm": perm.astype(NBF),
            "masks": masks.astype(NBF),
            "ones_red": ones_red,
            "ones_bc": ones_bc,
            "ident": ident,
        })

    res = run_bass_kernel_spmd(nc, in_maps, list(range(NCORES)),
                               trace=TRACE)
    LAST_EXEC_NS = res.exec_time_ns

    acc = res.results[0]["out"].astype(np.float32)
    for c in range(1, NCORES):
        acc = acc + res.results[c]["out"].astype(np.float32)
    acc *= 2.0 ** -(SO + SWO)
    # out[dc, sp2, p, k2*512 + col] -> out[(2*sp2+k2)*128 + p, dc*512 + col]
    out = (acc.reshape(8, 8, 128, 2, 512).transpose(1, 3, 2, 0, 4)
           .reshape(S, D))
    return np.ascontiguousarray(out).reshape(1, S, D)


# revision 12
# speedup vs baseline: 1.0762x; 1.0762x over previous
"""Tensor-parallel causal GQA self-attention (B=1, S=2048, D=4096, 32 q heads /
8 kv heads, HD=128, interleaved RoPE) on 8 trn2 NeuronCores.

Sharding: core c owns kv head c and q heads 4c..4c+3 (column-parallel
Wq/Wk/Wv, row-parallel Wo).  Each core computes a full [S, D] partial of the
output projection; the host sums the 8 partials (the "all-reduce").

fp8 DoubleRow strategy: fp8e4 + MatmulPerfMode.DoubleRow runs at 0.5
cycles/output-column and sums TWO K=128 products per instruction (4x the
fp32r MAC rate).  Heavy-tailed outputs make single-fp8 operands too lossy
anywhere in the main signal path, so the big GEMMs use a hi+lo split
(x = fp8(x) + fp8(x - fp8(x)), ~9 significand bits) in 3 passes
(xh@Wh + xl@Wh + xh@Wl) at 0.75x the fp32r cost:
  QKV     3-pass DoubleRow over k-tile pairs (hi/lo of x and W host-prepped)
  scores  stay bf16/fp32-rate (K=128, full precision; same cost as fp32r)
  P       exp(s-2.5) via ScalarE straight from PSUM to fp8e4 (e4m3 max 240,
          max score 7.36 -> max p 130); denominator sums the same quantized
          values so the softmax reweighting error mostly cancels
  AV      DoubleRow pairs of j-tiles, V in hi/lo (2 instrs per j-pair)
  denom   ones-matrix DoubleRow over the same j-pairs (1 instr per pair)
  o_proj  3-pass DoubleRow over head pairs, otn in hi/lo (bf16 staging)
Scale folding: Wq x2^10, Wk/Wv/Wo x2^7 (host), descaled via rope tables /
the denominator ones value (2^(7-2)) / a single host-side /512 of the
summed bf16 partials.
"""

import sys

if "/opt/trn_rl_repo" not in sys.path:
    sys.path.insert(0, "/opt/trn_rl_repo")

import numpy as np
import ml_dtypes

import concourse.bass as bass
import concourse.tile as tile
from concourse import bacc, mybir
from concourse.bass_utils import run_bass_kernel_spmd

S, D, NH, NKV, HD = 2048, 4096, 32, 8, 128
NCORES = 8
QH = NH // NCORES  # 4 q heads per core
ROPE_BASE = 500000.0

F32 = mybir.dt.float32
F32R = mybir.dt.float32r
BF16 = mybir.dt.bfloat16
E4 = mybir.dt.float8e4
AF = mybir.ActivationFunctionType
DR = mybir.MatmulPerfMode.DoubleRow

NE4 = ml_dtypes.float8_e4m3
NBF = ml_dtypes.bfloat16

SC = S // 512   # 4 s-chunks of 512
DP = D // 256   # 16 k-tile pairs
JT = S // 128   # 16 j-tiles of 128

SWQ = 10        # Wq*scale scaled by 2^10 before fp8
SWK = 7         # Wk, Wv, Wo scaled by 2^7
SWV = 7         # Wv host scale; V descaled by 2^(SVV-SWV) at the PSUM copy
SVV = 2         # V's effective fp8 scale (max |4v| ~ 19 < 240)
SWO = 7
SO = 2          # otn scaled by 2^2
CEXP = 2.5      # exp(s - CEXP); max causal score ~7.36 -> max p ~130 < 240

_CACHE = {}

TRACE = False
LAST_EXEC_NS = None


def _pair2(ap):
    """[128, 2*n] -> [128, 2, n] DoubleRow view."""
    n = ap.shape[-1] // 2
    return ap.rearrange("p (two n) -> p two n", two=2, n=n)


def _build_nc():
    nc = bacc.Bacc("TRN2", target_bir_lowering=False, debug=False,
                   num_devices=NCORES)

    xw_d = nc.declare_dram_parameter("xw", [SC, DP, 128, 3072], E4,
                                     isOutput=False)
    wq_d = nc.declare_dram_parameter("wq2", [DP, 128, 2048], E4,
                                     isOutput=False)
    wo_d = nc.declare_dram_parameter("wo4", [8, 128, 4096], E4,
                                     isOutput=False)
    cs_d = nc.declare_dram_parameter("cs4", [SC, HD, 2048], BF16,
                                     isOutput=False)
    perm_d = nc.declare_dram_parameter("perm", [HD, HD], BF16, isOutput=False)
    masks_d = nc.declare_dram_parameter("masks", [128, 4, 512], BF16,
                                        isOutput=False)
    onr_d = nc.declare_dram_parameter("ones_red", [128, 256], E4,
                                      isOutput=False)
    onb_d = nc.declare_dram_parameter("ones_bc", [1, 128], F32R,
                                      isOutput=False)
    ident_d = nc.declare_dram_parameter("ident", [HD, HD], BF16,
                                        isOutput=False)
    out_d = nc.declare_dram_parameter("out", [8, 8, 128, 1024], BF16,
                                      isOutput=True)

    with tile.TileContext(nc) as tc:
        from contextlib import ExitStack
        ctx = ExitStack()
        with ctx:
            wpool = ctx.enter_context(tc.tile_pool(name="wpool", bufs=16))
            xpool = ctx.enter_context(tc.tile_pool(name="xpool", bufs=3))
            qtp = ctx.enter_context(tc.tile_pool(name="qtp", bufs=9))
            otnp = ctx.enter_context(tc.tile_pool(name="otnp", bufs=16))
            tabp = ctx.enter_context(tc.tile_pool(name="tabp", bufs=2))
            ktp = ctx.enter_context(tc.tile_pool(name="ktp", bufs=4))
            vnp = ctx.enter_context(tc.tile_pool(name="vnp", bufs=8))
            stg = ctx.enter_context(tc.tile_pool(name="stg", bufs=4))
            rawp = ctx.enter_context(tc.tile_pool(name="rawp", bufs=6))
            ptp = ctx.enter_context(tc.tile_pool(name="ptp", bufs=28))
            mkp = ctx.enter_context(tc.tile_pool(name="mkp", bufs=1))
            cst = ctx.enter_context(tc.tile_pool(name="cst", bufs=1))
            rcp = ctx.enter_context(tc.tile_pool(name="rcp", bufs=4))
            ostp = ctx.enter_context(tc.tile_pool(name="ostp", bufs=3))
            psA = ctx.enter_context(
                tc.tile_pool(name="psA", bufs=6, space=bass.MemorySpace.PSUM))
            psB = ctx.enter_context(
                tc.tile_pool(name="psB", bufs=2, space=bass.MemorySpace.PSUM))

            perm_t = cst.tile([HD, HD], BF16, name="perm_t")
            ident_t = cst.tile([HD, HD], BF16, name="ident_t")
            onr_t = cst.tile([128, 256], E4, name="onr_t")
            onb_t = cst.tile([1, 128], F32R, name="onb_t")
            nexp_t = cst.tile([128, 1], F32, name="nexp_t")
            mask_t = mkp.tile([128, 4, 512], BF16, name="mask4")

            def table_loads():
                yield lambda: nc.sync.dma_start(perm_t[:], perm_d[:])
                yield lambda: nc.scalar.dma_start(ident_t[:], ident_d[:])
                yield lambda: nc.sync.dma_start(onr_t[:], onr_d[:])
                yield lambda: nc.scalar.dma_start(onb_t[:], onb_d[:])
                yield lambda: nc.sync.dma_start(mask_t[:], masks_d[:])
                yield lambda: nc.gpsimd.memset(nexp_t[:], -CEXP)

            wq_tiles = [None] * DP

            # persistent activations (bf16 for q/k, fp8 hi/lo pairs for V)
            QTc = [[qtp.tile([HD, 512], BF16, tag="qtc", name=f"qt{h}_{c}")
                    for c in range(SC)] for h in range(QH)]
            KTc = [ktp.tile([HD, 512], BF16, tag="ktc", name=f"kt{c}")
                   for c in range(SC)]
            Vhc = [vnp.tile([128, 4, 128], E4, tag="vnc", name=f"vh{c}")
                   for c in range(SC)]
            Vlc = [vnp.tile([128, 4, 128], E4, tag="vnc", name=f"vl{c}")
                   for c in range(SC)]

            def rope_copy(acc_ps, eng):
                raw = rawp.tile([128, 512], BF16, tag="raw", name="rope_raw")
                if eng == "act":
                    nc.scalar.activation(raw[:], acc_ps[:], AF.Copy)
                else:
                    nc.vector.tensor_copy(raw[:], acc_ps[:])
                return raw

            def rope_combine(raw, dest, cc, sn):
                rot = psB.tile([128, 512], F32, tag="tmp", name="rope_rot")
                nc.tensor.matmul(rot[:], perm_t[:], raw[:], start=True,
                                 stop=True)
                t1 = stg.tile([128, 512], BF16, tag="stg", name="rope_t1")
                nc.vector.tensor_mul(t1[:], raw[:], cc)
                t2 = stg.tile([128, 512], BF16, tag="stg", name="rope_t2")
                nc.vector.tensor_mul(t2[:], rot[:], sn)
                nc.vector.tensor_add(dest[:], t1[:], t2[:])

            boundary_pe = [None]

            def emit_boundary_pe():
                if boundary_pe[0] is not None:
                    boundary_pe[0]()
                    boundary_pe[0] = None

            cs_tiles = [None] * SC

            # otn hi/lo head-pair tiles: [c][hp] slot h%2
            otn_hi = [[None, None] for _ in range(SC)]
            otn_lo = [[None, None] for _ in range(SC)]
            tails = []

            def make_tail(c, h, ot, dsum):
                def tail():
                    dsg = rcp.tile([1, 512], F32R, tag="rc", name="dsg")
                    rc = rcp.tile([1, 512], F32R, tag="rc", name="rc")
                    with nc.allow_low_precision(reason="fp22 softmax recip"):
                        nc.vector.tensor_scalar_max(dsg[:], dsum[0:1, :],
                                                    1e-30)
                        nc.vector.reciprocal(rc[:], dsg[:])
                    bc = psA.tile([128, 512], F32, tag="acc", name="bc")
                    nc.tensor.matmul(bc[:], onb_t[:], rc[:], start=True,
                                     stop=True)
                    bcs = stg.tile([128, 512], BF16, tag="stg", name="bcs")
                    nc.scalar.activation(bcs[:], bc[:], AF.Copy)
                    ob = stg.tile([128, 512], BF16, tag="stg", name="otn_bf")
                    nc.vector.tensor_mul(ob[:], ot[:], bcs[:])
                    hi = otn_hi[c][h // 2][:, h % 2, :]
                    nc.vector.tensor_copy(hi, ob[:])
                    nc.vector.tensor_sub(otn_lo[c][h // 2][:, h % 2, :],
                                         ob[:], hi)
                return tail

            def score_pair(c, h, pr, pool):
                """Score + exp for j-tile pair pr of head h, chunk c."""
                qch = QTc[h][c][:]
                pp = ptp.tile([128, 2, 512], E4, tag="pt", name="pt")
                for jj in range(2):
                    jt = 2 * pr + jj
                    stp = pool.tile([128, 512], F32,
                                    tag="acc" if pool is psA else "tmp",
                                    name="stp")
                    nc.tensor.matmul(
                        stp[:],
                        KTc[jt // 4][:, 128 * (jt % 4):128 * (jt % 4 + 1)],
                        qch, start=True, stop=True)
                    pslot = pp[:, jj, :]
                    m = jt - 4 * c
                    if m >= 0:
                        # exp can exceed fp8 range above the diagonal;
                        # stage in bf16 so inf*0 never reaches pt
                        pe = stg.tile([128, 512], BF16, tag="stg",
                                      name="pe_t")
                        nc.scalar.activation(pe[:], stp[:], AF.Exp,
                                             bias=nexp_t[:])
                        nc.vector.tensor_mul(pslot, pe[:], mask_t[:, m, :])
                    else:
                        nc.scalar.activation(pslot, stp[:], AF.Exp,
                                             bias=nexp_t[:])
                return pp

            # pre[(h, pr)] -> pt pair tile, for the chunk whose scores were
            # interleaved into the following QKV dp-loop
            pre_pt = {}

            def attn_chunk(c):
                for hp in range(2):
                    otn_hi[c][hp] = otnp.tile([128, 2, 512], E4, tag="otn",
                                              name=f"oh{c}_{hp}")
                    otn_lo[c][hp] = otnp.tile([128, 2, 512], E4, tag="otn",
                                              name=f"ol{c}_{hp}")
                npair = 2 * c + 2
                for h in range(QH):
                    ot = psA.tile([128, 512], F32, tag="acc", name="ot_ps")
                    dsum = psB.tile([128, 512], F32, tag="tmp", name="dsum")
                    pairs = [None] * npair

                    def accum(pr, pp, ot=ot, dsum=dsum, npair=npair):
                        st = pr == 0
                        sp = pr == npair - 1
                        vh = Vhc[pr // 2][:, 2 * (pr % 2):2 * (pr % 2) + 2, :]
                        vl = Vlc[pr // 2][:, 2 * (pr % 2):2 * (pr % 2) + 2, :]
                        nc.tensor.matmul(ot[:], vh, pp[:], start=st,
                                         stop=False, perf_mode=DR)
                        nc.tensor.matmul(ot[:], vl, pp[:], start=False,
                                         stop=sp, perf_mode=DR)
                        nc.tensor.matmul(dsum[:], _pair2(onr_t[:]), pp[:],
                                         start=st, stop=sp, perf_mode=DR)

                    for pr in range(npair):
                        pairs[pr] = pre_pt.pop((h, pr), None)
                        if pairs[pr] is None:
                            pairs[pr] = score_pair(c, h, pr, psA)
                        if pr >= 2:
                            accum(pr - 2, pairs[pr - 2])
                    for k in (2, 1):
                        accum(npair - k, pairs[npair - k])
                    tails.append(make_tail(c, h, ot, dsum))
                    if len(tails) > 1:
                        tails.pop(0)()
                while tails:
                    tails.pop(0)()

            emit_attn = [attn_chunk]

            for sc in range(SC):
                qps = [psA.tile([128, 512], F32, tag="acc", name=f"qps{h}")
                       for h in range(QH)]
                kps = psA.tile([128, 512], F32, tag="acc", name="kps")
                vps = psA.tile([128, 512], F32, tag="acc", name="vps")
                # scores+exps of the previous chunk's attention are emitted
                # inside this dp loop (via the psB banks) so ACT works
                # through the exps while PE runs QKV
                pend = []
                if sc >= 1:
                    pend = [(h, pr) for h in range(QH)
                            for pr in range(2 * (sc - 1) + 2)]
                for dp in range(DP):
                    if sc == 0:
                        wt = wpool.tile([128, 2048], E4, tag="w",
                                        name=f"wq{dp}")
                        nc.scalar.dma_start(wt[:], wq_d[dp])
                        wq_tiles[dp] = wt
                    xt = xpool.tile([128, 3072], E4, tag="x", name="xt")
                    nc.sync.dma_start(xt[:], xw_d[sc, dp])
                    if sc == 0:
                        if dp == 0:
                            _tl = table_loads()
                        next(_tl, lambda: None)()
                    if dp == 1:
                        emit_boundary_pe()
                    if dp == 8:
                        cc4 = tabp.tile([128, 2048], BF16, tag="tab",
                                        name="cc4")
                        nc.sync.dma_start(cc4[:], cs_d[sc])
                        cs_tiles[sc] = cc4
                    xh = _pair2(xt[:, 0:1024])
                    xl = _pair2(xt[:, 1024:2048])
                    wkh = _pair2(xt[:, 2048:2304])
                    wkl = _pair2(xt[:, 2304:2560])
                    wvh = _pair2(xt[:, 2560:2816])
                    wvl = _pair2(xt[:, 2816:3072])
                    wqh = _pair2(wq_tiles[dp][:, 0:1024])
                    wql = _pair2(wq_tiles[dp][:, 1024:2048])
                    st = dp == 0
                    sp = dp == DP - 1
                    for h in range(QH):
                        hsl = slice(HD * h, HD * (h + 1))
                        nc.tensor.matmul(qps[h][:], wqh[:, :, hsl], xh[:],
                                         start=st, stop=False, perf_mode=DR)
                        nc.tensor.matmul(qps[h][:], wqh[:, :, hsl], xl[:],
                                         start=False, stop=False,
                                         perf_mode=DR)
                        nc.tensor.matmul(qps[h][:], wql[:, :, hsl], xh[:],
                                         start=False, stop=sp, perf_mode=DR)
                    nc.tensor.matmul(kps[:], wkh, xh[:], start=st,
                                     stop=False, perf_mode=DR)
                    nc.tensor.matmul(kps[:], wkh, xl[:], start=False,
                                     stop=False, perf_mode=DR)
                    nc.tensor.matmul(kps[:], wkl, xh[:], start=False,
                                     stop=sp, perf_mode=DR)
                    nc.tensor.matmul(vps[:], wvh, xh[:], start=st,
                                     stop=False, perf_mode=DR)
                    nc.tensor.matmul(vps[:], wvh, xl[:], start=False,
                                     stop=False, perf_mode=DR)
                    nc.tensor.matmul(vps[:], wvl, xh[:], start=False,
                                     stop=sp, perf_mode=DR)
                    if dp >= 2:
                        for _ in range(2):
                            if pend:
                                h, pr = pend.pop(0)
                                pre_pt[(h, pr)] = score_pair(sc - 1, h, pr,
                                                             psB)

                raw_k = rope_copy(kps, "act")
                vt_sb = rawp.tile([128, 512], BF16, tag="raw", name="vt_sb")
                nc.vector.tensor_scalar_mul(vt_sb[:], vps[:],
                                            2.0 ** (SVV - SWV))
                raw_q = [None] * QH
                raw_q[0] = rope_copy(qps[0], "dve")

                def boundary(sc=sc, raw_k=raw_k, vt_sb=vt_sb, raw_q=raw_q):
                    cc4 = cs_tiles[sc]
                    cq = cc4[:, 0:512]
                    sq = cc4[:, 512:1024]
                    ck = cc4[:, 1024:1536]
                    sk = cc4[:, 1536:2048]
                    rope_combine(raw_k, KTc[sc], ck, sk)
                    rope_combine(raw_q[0], QTc[0][sc], cq, sq)
                    for k4 in range(4):
                        vtp = psB.tile([128, 128], BF16, tag="tmp",
                                       name="vtp")
                        nc.tensor.transpose(
                            vtp[:], vt_sb[:, 128 * k4:128 * (k4 + 1)],
                            ident_t[:])
                        nc.scalar.activation(Vhc[sc][:, k4, :], vtp[:],
                                             AF.Copy)
                        nc.vector.tensor_sub(Vlc[sc][:, k4, :], vtp[:],
                                             Vhc[sc][:, k4, :])
                    for h in range(1, QH):
                        rope_combine(raw_q[h], QTc[h][sc], cq, sq)

                boundary_pe[0] = boundary
                if sc >= 1:
                    emit_attn[0](sc - 1)
                for h in range(1, QH):
                    raw_q[h] = rope_copy(qps[h],
                                         "act" if h % 2 == 0 else "dve")
                if sc == SC - 1:
                    wo_tiles = []
                    for dc in range(8):
                        wo = wpool.tile([128, 4096], E4, tag="w",
                                        name=f"wo{dc}")
                        nc.sync.dma_start(wo[:], wo_d[dc])
                        wo_tiles.append(wo)

                    emit_boundary_pe()
                    emit_attn[0](SC - 1)

            # ---- phase 3: output projection (row-parallel partial) ----
            def o_proj_all():
                for dc in range(8):
                    wo = wo_tiles[dc]
                    woh = [_pair2(wo[:, 0:1024]), _pair2(wo[:, 1024:2048])]
                    wol = [_pair2(wo[:, 2048:3072]), _pair2(wo[:, 3072:4096])]
                    for c in range(SC):
                        for lp in range(2):
                            ost = ostp.tile([128, 1024], BF16, tag="ost",
                                            name="ost")
                            for k2 in range(2):
                                kk = 2 * lp + k2
                                ksl = slice(128 * kk, 128 * (kk + 1))
                                acc = psA.tile([128, 512], F32, tag="acc",
                                               name="oacc")
                                for hp in range(2):
                                    oh = otn_hi[c][hp][:, :, ksl]
                                    ol = otn_lo[c][hp][:, :, ksl]
                                    nc.tensor.matmul(
                                        acc[:], oh, woh[hp][:],
                                        start=(hp == 0), stop=False,
                                        perf_mode=DR)
                                    nc.tensor.matmul(
                                        acc[:], ol, woh[hp][:],
                                        start=False, stop=False,
                                        perf_mode=DR)
                                    nc.tensor.matmul(
                                        acc[:], oh, wol[hp][:],
                                        start=False, stop=(hp == 1),
                                        perf_mode=DR)
                                dstc = ost[:, 512 * k2:512 * (k2 + 1)]
                                if k2 == 0:
                                    nc.vector.tensor_copy(dstc, acc[:])
                                else:
                                    nc.scalar.activation(dstc, acc[:],
                                                         AF.Copy)
                            eng = nc.sync if lp == 0 else nc.scalar
                            eng.dma_start(out_d[dc, 2 * c + lp], ost[:])

            o_proj_all()

    nc.compile()
    return nc


def _q8(x):
    return x.astype(NE4)


def _split8(x):
    h = x.astype(NE4)
    l = (x - h.astype(np.float32)).astype(NE4)
    return h, l


def _host_tables():
    pos = np.arange(S, dtype=np.float64)
    inv_freq = ROPE_BASE ** (-np.arange(0, HD, 2, dtype=np.float64) / HD)
    ang = np.outer(pos, inv_freq)  # [S, HD/2]
    cos = np.cos(ang).T.astype(np.float32)  # [HD/2, S]
    sin = np.sin(ang).T.astype(np.float32)
    cos2 = np.repeat(cos, 2, axis=0)  # [HD, S]
    sin2 = np.repeat(sin, 2, axis=0)
    sin2[0::2, :] *= -1.0  # even rows get -sin, odd rows +sin

    perm = np.zeros((HD, HD), dtype=np.float32)
    for i in range(HD):
        perm[i ^ 1, i] = 1.0

    masks = np.zeros((128, 4, 512), dtype=np.float32)
    jr = np.arange(128)[:, None]
    ir = np.arange(512)[None, :]
    for m in range(4):
        masks[:, m, :] = np.where(jr + 128 * m <= ir, 1.0, 0.0)

    return cos2, sin2, perm, masks


def kernel(x, Wq, Wk, Wv, Wo):
    global LAST_EXEC_NS
    if "nc" not in _CACHE:
        _CACHE["nc"] = _build_nc()
    nc = _CACHE["nc"]

    x = np.asarray(x, dtype=np.float32).reshape(S, D)
    Wq = np.asarray(Wq, dtype=np.float32)
    Wk = np.asarray(Wk, dtype=np.float32)
    Wv = np.asarray(Wv, dtype=np.float32)
    Wo = np.asarray(Wo, dtype=np.float32)

    xT = np.ascontiguousarray(x.T)  # [D, S]
    xh8, xl8 = _split8(xT)
    # [DP, 128, 2, 512] per (pair, row, slot, s-chunk col) built per chunk
    xh8r = xh8.reshape(DP, 2, 128, SC, 512)
    xl8r = xl8.reshape(DP, 2, 128, SC, 512)
    cos2, sin2, perm, masks = _host_tables()
    scale = np.float32(1.0 / np.sqrt(HD))

    # cs4[sc] = [cq | sq | ck | sk], each [HD, 512], bf16
    cs4 = np.empty((SC, HD, 2048), dtype=NBF)
    for scc in range(SC):
        cs = slice(512 * scc, 512 * (scc + 1))
        cs4[scc, :, 0:512] = (cos2[:, cs] * 2.0 ** -SWQ).astype(NBF)
        cs4[scc, :, 512:1024] = (sin2[:, cs] * 2.0 ** -SWQ).astype(NBF)
        cs4[scc, :, 1024:1536] = (cos2[:, cs] * 2.0 ** -SWK).astype(NBF)
        cs4[scc, :, 1536:2048] = (sin2[:, cs] * 2.0 ** -SWK).astype(NBF)

    ident = np.eye(HD, dtype=NBF)
    ones_red = np.zeros((128, 256), dtype=NE4)
    ones_red[:, 0] = 2.0 ** (SVV - SO)
    ones_red[:, 128] = 2.0 ** (SVV - SO)
    ones_bc = np.ones((1, 128), dtype=np.float32)

    in_maps = []
    for c in range(NCORES):
        qs = slice(QH * HD * c, QH * HD * (c + 1))
        ks = slice(HD * c, HD * (c + 1))
        wkh, wkl = _split8(Wk[:, ks] * 2.0 ** SWK)   # [D, 128] fp8
        wvh, wvl = _split8(Wv[:, ks] * 2.0 ** SWV)

        # xw[sc, dp] = [xh pair 1024 | xl pair 1024 | wkh 256 | wkl | wvh | wvl]
        xw = np.empty((SC, DP, 128, 3072), dtype=NE4)
        for scc in range(SC):
            xw[scc, :, :, 0:1024] = (
                xh8r[:, :, :, scc].transpose(0, 2, 1, 3).reshape(DP, 128, 1024))
            xw[scc, :, :, 1024:2048] = (
                xl8r[:, :, :, scc].transpose(0, 2, 1, 3).reshape(DP, 128, 1024))
        wk4 = np.stack([wkh, wkl, wvh, wvl], axis=0)  # [4, D, 128]
        wk4r = (wk4.reshape(4, DP, 2, 128, 128).transpose(1, 3, 0, 2, 4)
                .reshape(DP, 128, 1024))
        xw[:, :, :, 2048:3072] = wk4r[None]

        wqc = Wq[:, qs].astype(np.float32) * scale * 2.0 ** SWQ
        wqhh, wqll = _split8(wqc)   # [D, 512]
        wq2 = np.empty((DP, 128, 2048), dtype=NE4)
        wq2[:, :, 0:1024] = (wqhh.reshape(DP, 2, 128, 512)
                             .transpose(0, 2, 1, 3).reshape(DP, 128, 1024))
        wq2[:, :, 1024:2048] = (wqll.reshape(DP, 2, 128, 512)
                                .transpose(0, 2, 1, 3).reshape(DP, 128, 1024))

        # wo4[dc] = [Woh hp0 | Woh hp1 | Wol hp0 | Wol hp1], each [128,2,512]
        woc = Wo[qs, :].astype(np.float32) * 2.0 ** SWO  # [512, D]
        woh, wol = _split8(woc)
        wo4 = np.empty((8, 128, 4096), dtype=NE4)
        for part, w8 in ((0, woh), (1, wol)):
            # w8 [512, 4096] -> [4 heads, 128, 8 dc, 512]
            wr = w8.reshape(4, 128, 8, 512)
            for dc in range(8):
                for hp in range(2):
                    blk = wr[2 * hp:2 * hp + 2, :, dc, :]  # [2, 128, 512]
                    off = 2048 * part + 1024 * hp
                    wo4[dc, :, off:off + 1024] = (
                        blk.transpose(1, 0, 2).reshape(128, 1024))

        in_maps.append({
            "xw": xw,
            "wq2": wq2,
            "wo4": wo4,
            "cs4": cs4,
            "perm": perm.astype(NBF),
            "masks": masks.astype(NBF),
            "ones_red": ones_red,
            "ones_bc": ones_bc,
            "ident": ident,
        })

    res = run_bass_kernel_spmd(nc, in_maps, list(range(NCORES)),
                               trace=TRACE)
    LAST_EXEC_NS = res.exec_time_ns

    acc = res.results[0]["out"].astype(np.float32)
    for c in range(1, NCORES):
        acc = acc + res.results[c]["out"].astype(np.float32)
    acc *= 2.0 ** -(SO + SWO)
    # out[dc, sp2, p, k2*512 + col] -> out[(2*sp2+k2)*128 + p, dc*512 + col]
    out = (acc.reshape(8, 8, 128, 2, 512).transpose(1, 3, 2, 0, 4)
           .reshape(S, D))
    return np.ascontiguousarray(out).reshape(1, S, D)


# revision 14
# speedup vs baseline: 1.0981x; 1.0203x over previous
"""Tensor-parallel causal GQA self-attention (B=1, S=2048, D=4096, 32 q heads /
8 kv heads, HD=128, interleaved RoPE) on 8 trn2 NeuronCores.

Sharding: core c owns kv head c and q heads 4c..4c+3 (column-parallel
Wq/Wk/Wv, row-parallel Wo).  Each core computes a full [S, D] partial of the
output projection; the host sums the 8 partials (the "all-reduce").

fp8 DoubleRow strategy: fp8e4 + MatmulPerfMode.DoubleRow runs at 0.5
cycles/output-column and sums TWO K=128 products per instruction (4x the
fp32r MAC rate).  Heavy-tailed outputs make single-fp8 operands too lossy
anywhere in the main signal path, so the big GEMMs use a hi+lo split
(x = fp8(x) + fp8(x - fp8(x)), ~9 significand bits) in 3 passes
(xh@Wh + xl@Wh + xh@Wl) at 0.75x the fp32r cost:
  QKV     3-pass DoubleRow over k-tile pairs (hi/lo of x and W host-prepped)
  scores  stay bf16/fp32-rate (K=128, full precision; same cost as fp32r)
  P       exp(s-2.5) via ScalarE straight from PSUM to fp8e4 (e4m3 max 240,
          max score 7.36 -> max p 130); denominator sums the same quantized
          values so the softmax reweighting error mostly cancels
  AV      DoubleRow pairs of j-tiles, V in hi/lo (2 instrs per j-pair)
  denom   ones-matrix DoubleRow over the same j-pairs (1 instr per pair)
  o_proj  3-pass DoubleRow over head pairs, otn in hi/lo (bf16 staging)
Scale folding: Wq x2^10, Wk/Wv/Wo x2^7 (host), descaled via rope tables /
the denominator ones value (2^(7-2)) / a single host-side /512 of the
summed bf16 partials.
"""

import sys

if "/opt/trn_rl_repo" not in sys.path:
    sys.path.insert(0, "/opt/trn_rl_repo")

import numpy as np
import ml_dtypes

import concourse.bass as bass
import concourse.tile as tile
from concourse import bacc, mybir
from concourse.bass_utils import run_bass_kernel_spmd

S, D, NH, NKV, HD = 2048, 4096, 32, 8, 128
NCORES = 8
QH = NH // NCORES  # 4 q heads per core
ROPE_BASE = 500000.0

F32 = mybir.dt.float32
F32R = mybir.dt.float32r
BF16 = mybir.dt.bfloat16
E4 = mybir.dt.float8e4
AF = mybir.ActivationFunctionType
DR = mybir.MatmulPerfMode.DoubleRow

NE4 = ml_dtypes.float8_e4m3
NBF = ml_dtypes.bfloat16

SC = S // 512   # 4 s-chunks of 512
DP = D // 256   # 16 k-tile pairs
JT = S // 128   # 16 j-tiles of 128

SWQ = 10        # Wq*scale scaled by 2^10 before fp8
SWK = 7         # Wk, Wv, Wo scaled by 2^7
SWV = 7         # Wv host scale; V descaled by 2^(SVV-SWV) at the PSUM copy
SVV = 2         # V's effective fp8 scale (max |4v| ~ 19 < 240)
SWO = 7
SO = 2          # otn scaled by 2^2
CEXP = 2.5      # exp(s - CEXP); max causal score ~7.36 -> max p ~130 < 240

_CACHE = {}

TRACE = False
LAST_EXEC_NS = None


def _pair2(ap):
    """[128, 2*n] -> [128, 2, n] DoubleRow view."""
    n = ap.shape[-1] // 2
    return ap.rearrange("p (two n) -> p two n", two=2, n=n)


def _build_nc():
    nc = bacc.Bacc("TRN2", target_bir_lowering=False, debug=False,
                   num_devices=NCORES)

    xw_d = nc.declare_dram_parameter("xw", [SC, DP, 128, 3072], E4,
                                     isOutput=False)
    wq_d = nc.declare_dram_parameter("wq2", [DP, 128, 2048], E4,
                                     isOutput=False)
    wo_d = nc.declare_dram_parameter("wo4", [8, 128, 4096], E4,
                                     isOutput=False)
    cs_d = nc.declare_dram_parameter("cs4", [SC, HD, 2048], BF16,
                                     isOutput=False)
    perm_d = nc.declare_dram_parameter("perm", [HD, HD], BF16, isOutput=False)
    masks_d = nc.declare_dram_parameter("masks", [128, 4, 512], BF16,
                                        isOutput=False)
    onr_d = nc.declare_dram_parameter("ones_red", [128, 256], E4,
                                      isOutput=False)
    onb_d = nc.declare_dram_parameter("ones_bc", [1, 128], F32R,
                                      isOutput=False)
    ident_d = nc.declare_dram_parameter("ident", [HD, HD], BF16,
                                        isOutput=False)
    out_d = nc.declare_dram_parameter("out", [8, 8, 128, 1024], BF16,
                                      isOutput=True)

    with tile.TileContext(nc) as tc:
        from contextlib import ExitStack
        ctx = ExitStack()
        with ctx:
            wpool = ctx.enter_context(tc.tile_pool(name="wpool", bufs=16))
            xpool = ctx.enter_context(tc.tile_pool(name="xpool", bufs=3))
            qtp = ctx.enter_context(tc.tile_pool(name="qtp", bufs=9))
            otnp = ctx.enter_context(tc.tile_pool(name="otnp", bufs=16))
            tabp = ctx.enter_context(tc.tile_pool(name="tabp", bufs=2))
            ktp = ctx.enter_context(tc.tile_pool(name="ktp", bufs=4))
            vnp = ctx.enter_context(tc.tile_pool(name="vnp", bufs=8))
            stg = ctx.enter_context(tc.tile_pool(name="stg", bufs=4))
            rawp = ctx.enter_context(tc.tile_pool(name="rawp", bufs=6))
            ptp = ctx.enter_context(tc.tile_pool(name="ptp", bufs=40))
            mkp = ctx.enter_context(tc.tile_pool(name="mkp", bufs=1))
            cst = ctx.enter_context(tc.tile_pool(name="cst", bufs=1))
            rcp = ctx.enter_context(tc.tile_pool(name="rcp", bufs=4))
            ostp = ctx.enter_context(tc.tile_pool(name="ostp", bufs=3))
            psA = ctx.enter_context(
                tc.tile_pool(name="psA", bufs=6, space=bass.MemorySpace.PSUM))
            psB = ctx.enter_context(
                tc.tile_pool(name="psB", bufs=2, space=bass.MemorySpace.PSUM))

            perm_t = cst.tile([HD, HD], BF16, name="perm_t")
            ident_t = cst.tile([HD, HD], BF16, name="ident_t")
            onr_t = cst.tile([128, 256], E4, name="onr_t")
            onb_t = cst.tile([1, 128], F32R, name="onb_t")
            nexp_t = cst.tile([128, 1], F32, name="nexp_t")
            mask_t = mkp.tile([128, 4, 512], BF16, name="mask4")

            def table_loads():
                yield lambda: nc.sync.dma_start(perm_t[:], perm_d[:])
                yield lambda: nc.scalar.dma_start(ident_t[:], ident_d[:])
                yield lambda: nc.sync.dma_start(onr_t[:], onr_d[:])
                yield lambda: nc.scalar.dma_start(onb_t[:], onb_d[:])
                yield lambda: nc.sync.dma_start(mask_t[:], masks_d[:])
                yield lambda: nc.gpsimd.memset(nexp_t[:], -CEXP)

            wq_tiles = [None] * DP

            # persistent activations (bf16 for q/k, fp8 hi/lo pairs for V)
            QTc = [[qtp.tile([HD, 512], BF16, tag="qtc", name=f"qt{h}_{c}")
                    for c in range(SC)] for h in range(QH)]
            KTc = [ktp.tile([HD, 512], BF16, tag="ktc", name=f"kt{c}")
                   for c in range(SC)]
            Vhc = [vnp.tile([128, 4, 128], E4, tag="vnc", name=f"vh{c}")
                   for c in range(SC)]
            Vlc = [vnp.tile([128, 4, 128], E4, tag="vnc", name=f"vl{c}")
                   for c in range(SC)]

            def rope_copy(acc_ps, eng):
                raw = rawp.tile([128, 512], BF16, tag="raw", name="rope_raw")
                if eng == "act":
                    nc.scalar.activation(raw[:], acc_ps[:], AF.Copy)
                else:
                    nc.vector.tensor_copy(raw[:], acc_ps[:])
                return raw

            def rope_combine(raw, dest, cc, sn):
                rot = psB.tile([128, 512], F32, tag="tmp", name="rope_rot")
                nc.tensor.matmul(rot[:], perm_t[:], raw[:], start=True,
                                 stop=True)
                t1 = stg.tile([128, 512], BF16, tag="stg", name="rope_t1")
                nc.vector.tensor_mul(t1[:], raw[:], cc)
                t2 = stg.tile([128, 512], BF16, tag="stg", name="rope_t2")
                nc.vector.tensor_mul(t2[:], rot[:], sn)
                nc.vector.tensor_add(dest[:], t1[:], t2[:])

            boundary_pe = [None]

            def emit_boundary_pe():
                if boundary_pe[0] is not None:
                    boundary_pe[0]()
                    boundary_pe[0] = None

            cs_tiles = [None] * SC

            # otn hi/lo head-pair tiles: [c][hp] slot h%2
            otn_hi = [[None, None] for _ in range(SC)]
            otn_lo = [[None, None] for _ in range(SC)]
            tails = []

            def make_tail(c, h, ot, dsum):
                def tail():
                    dsg = rcp.tile([1, 512], F32R, tag="rc", name="dsg")
                    rc = rcp.tile([1, 512], F32R, tag="rc", name="rc")
                    with nc.allow_low_precision(reason="fp22 softmax recip"):
                        nc.vector.tensor_scalar_max(dsg[:], dsum[0:1, :],
                                                    1e-30)
                        nc.vector.reciprocal(rc[:], dsg[:])
                    bc = psA.tile([128, 512], F32, tag="acc", name="bc")
                    nc.tensor.matmul(bc[:], onb_t[:], rc[:], start=True,
                                     stop=True)
                    bcs = stg.tile([128, 512], BF16, tag="stg", name="bcs")
                    nc.scalar.activation(bcs[:], bc[:], AF.Copy)
                    ob = stg.tile([128, 512], BF16, tag="stg", name="otn_bf")
                    nc.vector.tensor_mul(ob[:], ot[:], bcs[:])
                    hi = otn_hi[c][h // 2][:, h % 2, :]
                    nc.vector.tensor_copy(hi, ob[:])
                    nc.vector.tensor_sub(otn_lo[c][h // 2][:, h % 2, :],
                                         ob[:], hi)
                return tail

            def score_pair(c, h, pr, pool):
                """Score + exp for j-tile pair pr of head h, chunk c."""
                qch = QTc[h][c][:]
                pp = ptp.tile([128, 2, 512], E4, tag="pt", name="pt")
                for jj in range(2):
                    jt = 2 * pr + jj
                    stp = pool.tile([128, 512], F32,
                                    tag="acc" if pool is psA else "tmp",
                                    name="stp")
                    nc.tensor.matmul(
                        stp[:],
                        KTc[jt // 4][:, 128 * (jt % 4):128 * (jt % 4 + 1)],
                        qch, start=True, stop=True)
                    pslot = pp[:, jj, :]
                    m = jt - 4 * c
                    if m >= 0:
                        # exp can exceed fp8 range above the diagonal;
                        # stage in bf16 so inf*0 never reaches pt
                        pe = stg.tile([128, 512], BF16, tag="stg",
                                      name="pe_t")
                        nc.scalar.activation(pe[:], stp[:], AF.Exp,
                                             bias=nexp_t[:])
                        nc.vector.tensor_mul(pslot, pe[:], mask_t[:, m, :])
                    else:
                        nc.scalar.activation(pslot, stp[:], AF.Exp,
                                             bias=nexp_t[:])
                return pp

            # pre[(h, pr)] -> pt pair tile, for the chunk whose scores were
            # interleaved into the following QKV dp-loop
            pre_pt = {}

            def attn_chunk(c):
                for hp in range(2):
                    otn_hi[c][hp] = otnp.tile([128, 2, 512], E4, tag="otn",
                                              name=f"oh{c}_{hp}")
                    otn_lo[c][hp] = otnp.tile([128, 2, 512], E4, tag="otn",
                                              name=f"ol{c}_{hp}")
                npair = 2 * c + 2
                for h in range(QH):
                    ot = psA.tile([128, 512], F32, tag="acc", name="ot_ps")
                    dsum = psB.tile([128, 512], F32, tag="tmp", name="dsum")
                    pairs = [None] * npair

                    def accum(pr, pp, ot=ot, dsum=dsum, npair=npair):
                        st = pr == 0
                        sp = pr == npair - 1
                        vh = Vhc[pr // 2][:, 2 * (pr % 2):2 * (pr % 2) + 2, :]
                        vl = Vlc[pr // 2][:, 2 * (pr % 2):2 * (pr % 2) + 2, :]
                        nc.tensor.matmul(ot[:], vh, pp[:], start=st,
                                         stop=False, perf_mode=DR)
                        nc.tensor.matmul(ot[:], vl, pp[:], start=False,
                                         stop=sp, perf_mode=DR)
                        nc.tensor.matmul(dsum[:], _pair2(onr_t[:]), pp[:],
                                         start=st, stop=sp, perf_mode=DR)

                    for pr in range(npair):
                        pairs[pr] = pre_pt.pop((h, pr), None)
                        if pairs[pr] is None:
                            pairs[pr] = score_pair(c, h, pr, psA)
                        if pr >= 2:
                            accum(pr - 2, pairs[pr - 2])
                    for k in (2, 1):
                        accum(npair - k, pairs[npair - k])
                    tails.append(make_tail(c, h, ot, dsum))
                    if len(tails) > 1:
                        tails.pop(0)()
                while tails:
                    tails.pop(0)()

            emit_attn = [attn_chunk]

            for sc in range(SC):
                qps = [psA.tile([128, 512], F32, tag="acc", name=f"qps{h}")
                       for h in range(QH)]
                kps = psA.tile([128, 512], F32, tag="acc", name="kps")
                vps = psA.tile([128, 512], F32, tag="acc", name="vps")
                # scores+exps of the previous chunk's attention are emitted
                # inside this dp loop (via the psB banks) so ACT works
                # through the exps while PE runs QKV
                pend = []
                if sc >= 1:
                    pend = [(h, pr) for h in range(QH)
                            for pr in range(2 * (sc - 1) + 2)]
                for dp in range(DP):
                    if sc == 0:
                        wt = wpool.tile([128, 2048], E4, tag="w",
                                        name=f"wq{dp}")
                        nc.scalar.dma_start(wt[:], wq_d[dp])
                        wq_tiles[dp] = wt
                    xt = xpool.tile([128, 3072], E4, tag="x", name="xt")
                    nc.sync.dma_start(xt[:], xw_d[sc, dp])
                    if sc == 0:
                        if dp == 0:
                            _tl = table_loads()
                        next(_tl, lambda: None)()
                    if dp == 1:
                        emit_boundary_pe()
                    if dp == 8:
                        cc4 = tabp.tile([128, 2048], BF16, tag="tab",
                                        name="cc4")
                        nc.sync.dma_start(cc4[:], cs_d[sc])
                        cs_tiles[sc] = cc4
                    xh = _pair2(xt[:, 0:1024])
                    xl = _pair2(xt[:, 1024:2048])
                    wkh = _pair2(xt[:, 2048:2304])
                    wkl = _pair2(xt[:, 2304:2560])
                    wvh = _pair2(xt[:, 2560:2816])
                    wvl = _pair2(xt[:, 2816:3072])
                    wqh = _pair2(wq_tiles[dp][:, 0:1024])
                    wql = _pair2(wq_tiles[dp][:, 1024:2048])
                    st = dp == 0
                    sp = dp == DP - 1
                    for h in range(QH):
                        hsl = slice(HD * h, HD * (h + 1))
                        nc.tensor.matmul(qps[h][:], wqh[:, :, hsl], xh[:],
                                         start=st, stop=False, perf_mode=DR)
                        nc.tensor.matmul(qps[h][:], wqh[:, :, hsl], xl[:],
                                         start=False, stop=False,
                                         perf_mode=DR)
                        nc.tensor.matmul(qps[h][:], wql[:, :, hsl], xh[:],
                                         start=False, stop=sp, perf_mode=DR)
                    nc.tensor.matmul(kps[:], wkh, xh[:], start=st,
                                     stop=False, perf_mode=DR)
                    nc.tensor.matmul(kps[:], wkh, xl[:], start=False,
                                     stop=False, perf_mode=DR)
                    nc.tensor.matmul(kps[:], wkl, xh[:], start=False,
                                     stop=sp, perf_mode=DR)
                    nc.tensor.matmul(vps[:], wvh, xh[:], start=st,
                                     stop=False, perf_mode=DR)
                    nc.tensor.matmul(vps[:], wvh, xl[:], start=False,
                                     stop=False, perf_mode=DR)
                    nc.tensor.matmul(vps[:], wvl, xh[:], start=False,
                                     stop=sp, perf_mode=DR)
                    if dp >= 2:
                        for _ in range(2):
                            if pend:
                                h, pr = pend.pop(0)
                                pre_pt[(h, pr)] = score_pair(sc - 1, h, pr,
                                                             psB)

                raw_k = rope_copy(kps, "act")
                vt_sb = rawp.tile([128, 512], BF16, tag="raw", name="vt_sb")
                nc.vector.tensor_scalar_mul(vt_sb[:], vps[:],
                                            2.0 ** (SVV - SWV))
                raw_q = [None] * QH
                raw_q[0] = rope_copy(qps[0], "dve")

                def boundary(sc=sc, raw_k=raw_k, vt_sb=vt_sb, raw_q=raw_q):
                    cc4 = cs_tiles[sc]
                    cq = cc4[:, 0:512]
                    sq = cc4[:, 512:1024]
                    ck = cc4[:, 1024:1536]
                    sk = cc4[:, 1536:2048]
                    rope_combine(raw_k, KTc[sc], ck, sk)
                    rope_combine(raw_q[0], QTc[0][sc], cq, sq)
                    for k4 in range(4):
                        vtp = psB.tile([128, 128], BF16, tag="tmp",
                                       name="vtp")
                        nc.tensor.transpose(
                            vtp[:], vt_sb[:, 128 * k4:128 * (k4 + 1)],
                            ident_t[:])
                        nc.scalar.activation(Vhc[sc][:, k4, :], vtp[:],
                                             AF.Copy)
                        nc.vector.tensor_sub(Vlc[sc][:, k4, :], vtp[:],
                                             Vhc[sc][:, k4, :])
                    for h in range(1, QH):
                        rope_combine(raw_q[h], QTc[h][sc], cq, sq)

                boundary_pe[0] = boundary
                if sc >= 1:
                    emit_attn[0](sc - 1)
                for h in range(1, QH):
                    raw_q[h] = rope_copy(qps[h],
                                         "act" if h % 2 == 0 else "dve")
                if sc == SC - 1:
                    wo_tiles = []
                    for dc in range(8):
                        wo = wpool.tile([128, 4096], E4, tag="w",
                                        name=f"wo{dc}")
                        nc.sync.dma_start(wo[:], wo_d[dc])
                        wo_tiles.append(wo)

                    emit_boundary_pe()

            # ---- phase 3: output projection (row-parallel partial), with
            # the last attention chunk's scores/exps interleaved so its exp
            # cost hides under o_proj's PE work
            def o_block(dc, c):
                wo = wo_tiles[dc]
                woh = [_pair2(wo[:, 0:1024]), _pair2(wo[:, 1024:2048])]
                wol = [_pair2(wo[:, 2048:3072]), _pair2(wo[:, 3072:4096])]
                for lp in range(2):
                    ost = ostp.tile([128, 1024], BF16, tag="ost",
                                    name="ost")
                    for k2 in range(2):
                        kk = 2 * lp + k2
                        ksl = slice(128 * kk, 128 * (kk + 1))
                        acc = psA.tile([128, 512], F32, tag="acc",
                                       name="oacc")
                        for hp in range(2):
                            oh = otn_hi[c][hp][:, :, ksl]
                            ol = otn_lo[c][hp][:, :, ksl]
                            nc.tensor.matmul(acc[:], oh, woh[hp][:],
                                             start=(hp == 0), stop=False,
                                             perf_mode=DR)
                            nc.tensor.matmul(acc[:], ol, woh[hp][:],
                                             start=False, stop=False,
                                             perf_mode=DR)
                            nc.tensor.matmul(acc[:], oh, wol[hp][:],
                                             start=False, stop=(hp == 1),
                                             perf_mode=DR)
                        dstc = ost[:, 512 * k2:512 * (k2 + 1)]
                        if k2 == 0:
                            nc.vector.tensor_copy(dstc, acc[:])
                        else:
                            nc.scalar.activation(dstc, acc[:], AF.Copy)
                    eng = nc.sync if lp == 0 else nc.scalar
                    eng.dma_start(out_d[dc, 2 * c + lp], ost[:])

            def o_proj_all():
                pend3 = [(h, pr) for h in range(QH)
                         for pr in range(2 * (SC - 1) + 2)]
                for c in range(SC - 1):
                    for dc in range(8):
                        o_block(dc, c)
                        for _ in range(2):
                            if pend3:
                                h, pr = pend3.pop(0)
                                pre_pt[(h, pr)] = score_pair(SC - 1, h, pr,
                                                             psB)
                emit_attn[0](SC - 1)
                for dc in range(8):
                    o_block(dc, SC - 1)

            o_proj_all()

    nc.compile()
    return nc


def _q8(x):
    return x.astype(NE4)


def _split8(x):
    h = x.astype(NE4)
    l = (x - h.astype(np.float32)).astype(NE4)
    return h, l


def _host_tables():
    pos = np.arange(S, dtype=np.float64)
    inv_freq = ROPE_BASE ** (-np.arange(0, HD, 2, dtype=np.float64) / HD)
    ang = np.outer(pos, inv_freq)  # [S, HD/2]
    cos = np.cos(ang).T.astype(np.float32)  # [HD/2, S]
    sin = np.sin(ang).T.astype(np.float32)
    cos2 = np.repeat(cos, 2, axis=0)  # [HD, S]
    sin2 = np.repeat(sin, 2, axis=0)
    sin2[0::2, :] *= -1.0  # even rows get -sin, odd rows +sin

    perm = np.zeros((HD, HD), dtype=np.float32)
    for i in range(HD):
        perm[i ^ 1, i] = 1.0

    masks = np.zeros((128, 4, 512), dtype=np.float32)
    jr = np.arange(128)[:, None]
    ir = np.arange(512)[None, :]
    for m in range(4):
        masks[:, m, :] = np.where(jr + 128 * m <= ir, 1.0, 0.0)

    return cos2, sin2, perm, masks


def kernel(x, Wq, Wk, Wv, Wo):
    global LAST_EXEC_NS
    if "nc" not in _CACHE:
        _CACHE["nc"] = _build_nc()
    nc = _CACHE["nc"]

    x = np.asarray(x, dtype=np.float32).reshape(S, D)
    Wq = np.asarray(Wq, dtype=np.float32)
    Wk = np.asarray(Wk, dtype=np.float32)
    Wv = np.asarray(Wv, dtype=np.float32)
    Wo = np.asarray(Wo, dtype=np.float32)

    xT = np.ascontiguousarray(x.T)  # [D, S]
    xh8, xl8 = _split8(xT)
    # [DP, 128, 2, 512] per (pair, row, slot, s-chunk col) built per chunk
    xh8r = xh8.reshape(DP, 2, 128, SC, 512)
    xl8r = xl8.reshape(DP, 2, 128, SC, 512)
    cos2, sin2, perm, masks = _host_tables()
    scale = np.float32(1.0 / np.sqrt(HD))

    # cs4[sc] = [cq | sq | ck | sk], each [HD, 512], bf16
    cs4 = np.empty((SC, HD, 2048), dtype=NBF)
    for scc in range(SC):
        cs = slice(512 * scc, 512 * (scc + 1))
        cs4[scc, :, 0:512] = (cos2[:, cs] * 2.0 ** -SWQ).astype(NBF)
        cs4[scc, :, 512:1024] = (sin2[:, cs] * 2.0 ** -SWQ).astype(NBF)
        cs4[scc, :, 1024:1536] = (cos2[:, cs] * 2.0 ** -SWK).astype(NBF)
        cs4[scc, :, 1536:2048] = (sin2[:, cs] * 2.0 ** -SWK).astype(NBF)

    ident = np.eye(HD, dtype=NBF)
    ones_red = np.zeros((128, 256), dtype=NE4)
    ones_red[:, 0] = 2.0 ** (SVV - SO)
    ones_red[:, 128] = 2.0 ** (SVV - SO)
    ones_bc = np.ones((1, 128), dtype=np.float32)

    in_maps = []
    for c in range(NCORES):
        qs = slice(QH * HD * c, QH * HD * (c + 1))
        ks = slice(HD * c, HD * (c + 1))
        wkh, wkl = _split8(Wk[:, ks] * 2.0 ** SWK)   # [D, 128] fp8
        wvh, wvl = _split8(Wv[:, ks] * 2.0 ** SWV)

        # xw[sc, dp] = [xh pair 1024 | xl pair 1024 | wkh 256 | wkl | wvh | wvl]
        xw = np.empty((SC, DP, 128, 3072), dtype=NE4)
        for scc in range(SC):
            xw[scc, :, :, 0:1024] = (
                xh8r[:, :, :, scc].transpose(0, 2, 1, 3).reshape(DP, 128, 1024))
            xw[scc, :, :, 1024:2048] = (
                xl8r[:, :, :, scc].transpose(0, 2, 1, 3).reshape(DP, 128, 1024))
        wk4 = np.stack([wkh, wkl, wvh, wvl], axis=0)  # [4, D, 128]
        wk4r = (wk4.reshape(4, DP, 2, 128, 128).transpose(1, 3, 0, 2, 4)
                .reshape(DP, 128, 1024))
        xw[:, :, :, 2048:3072] = wk4r[None]

        wqc = Wq[:, qs].astype(np.float32) * scale * 2.0 ** SWQ
        wqhh, wqll = _split8(wqc)   # [D, 512]
        wq2 = np.empty((DP, 128, 2048), dtype=NE4)
        wq2[:, :, 0:1024] = (wqhh.reshape(DP, 2, 128, 512)
                             .transpose(0, 2, 1, 3).reshape(DP, 128, 1024))
        wq2[:, :, 1024:2048] = (wqll.reshape(DP, 2, 128, 512)
                                .transpose(0, 2, 1, 3).reshape(DP, 128, 1024))

        # wo4[dc] = [Woh hp0 | Woh hp1 | Wol hp0 | Wol hp1], each [128,2,512]
        woc = Wo[qs, :].astype(np.float32) * 2.0 ** SWO  # [512, D]
        woh, wol = _split8(woc)
        wo4 = np.empty((8, 128, 4096), dtype=NE4)
        for part, w8 in ((0, woh), (1, wol)):
            # w8 [512, 4096] -> [4 heads, 128, 8 dc, 512]
            wr = w8.reshape(4, 128, 8, 512)
            for dc in range(8):
                for hp in range(2):
                    blk = wr[2 * hp:2 * hp + 2, :, dc, :]  # [2, 128, 512]
                    off = 2048 * part + 1024 * hp
                    wo4[dc, :, off:off + 1024] = (
                        blk.transpose(1, 0, 2).reshape(128, 1024))

        in_maps.append({
            "xw": xw,
            "wq2": wq2,
            "wo4": wo4,
            "cs4": cs4,
            "perm": perm.astype(NBF),
            "masks": masks.astype(NBF),
            "ones_red": ones_red,
            "ones_bc": ones_bc,
            "ident": ident,
        })

    res = run_bass_kernel_spmd(nc, in_maps, list(range(NCORES)),
                               trace=TRACE)
    LAST_EXEC_NS = res.exec_time_ns

    acc = res.results[0]["out"].astype(np.float32)
    for c in range(1, NCORES):
        acc = acc + res.results[c]["out"].astype(np.float32)
    acc *= 2.0 ** -(SO + SWO)
    # out[dc, sp2, p, k2*512 + col] -> out[(2*sp2+k2)*128 + p, dc*512 + col]
    out = (acc.reshape(8, 8, 128, 2, 512).transpose(1, 3, 2, 0, 4)
           .reshape(S, D))
    return np.ascontiguousarray(out).reshape(1, S, D)


# revision 16
# speedup vs baseline: 1.0985x; 1.0003x over previous
"""Tensor-parallel causal GQA self-attention (B=1, S=2048, D=4096, 32 q heads /
8 kv heads, HD=128, interleaved RoPE) on 8 trn2 NeuronCores.

Sharding: core c owns kv head c and q heads 4c..4c+3 (column-parallel
Wq/Wk/Wv, row-parallel Wo).  Each core computes a full [S, D] partial of the
output projection; the host sums the 8 partials (the "all-reduce").

fp8 DoubleRow strategy: fp8e4 + MatmulPerfMode.DoubleRow runs at 0.5
cycles/output-column and sums TWO K=128 products per instruction (4x the
fp32r MAC rate).  Heavy-tailed outputs make single-fp8 operands too lossy
anywhere in the main signal path, so the big GEMMs use a hi+lo split
(x = fp8(x) + fp8(x - fp8(x)), ~9 significand bits) in 3 passes
(xh@Wh + xl@Wh + xh@Wl) at 0.75x the fp32r cost:
  QKV     3-pass DoubleRow over k-tile pairs (hi/lo of x and W host-prepped)
  scores  stay bf16/fp32-rate (K=128, full precision; same cost as fp32r)
  P       exp(s-2.5) via ScalarE straight from PSUM to fp8e4 (e4m3 max 240,
          max score 7.36 -> max p 130); denominator sums the same quantized
          values so the softmax reweighting error mostly cancels
  AV      DoubleRow pairs of j-tiles, V in hi/lo (2 instrs per j-pair)
  denom   ones-matrix DoubleRow over the same j-pairs (1 instr per pair)
  o_proj  3-pass DoubleRow over head pairs, otn in hi/lo (bf16 staging)
Scale folding: Wq x2^10, Wk/Wv/Wo x2^7 (host), descaled via rope tables /
the denominator ones value (2^(7-2)) / a single host-side /512 of the
summed bf16 partials.
"""

import sys

if "/opt/trn_rl_repo" not in sys.path:
    sys.path.insert(0, "/opt/trn_rl_repo")

import numpy as np
import ml_dtypes

import concourse.bass as bass
import concourse.tile as tile
from concourse import bacc, mybir
from concourse.bass_utils import run_bass_kernel_spmd

S, D, NH, NKV, HD = 2048, 4096, 32, 8, 128
NCORES = 8
QH = NH // NCORES  # 4 q heads per core
ROPE_BASE = 500000.0

F32 = mybir.dt.float32
F32R = mybir.dt.float32r
BF16 = mybir.dt.bfloat16
E4 = mybir.dt.float8e4
AF = mybir.ActivationFunctionType
DR = mybir.MatmulPerfMode.DoubleRow

NE4 = ml_dtypes.float8_e4m3
NBF = ml_dtypes.bfloat16

SC = S // 512   # 4 s-chunks of 512
DP = D // 256   # 16 k-tile pairs
JT = S // 128   # 16 j-tiles of 128

SWQ = 10        # Wq*scale scaled by 2^10 before fp8
SWK = 7         # Wk, Wv, Wo scaled by 2^7
SWV = 7         # Wv host scale; V descaled by 2^(SVV-SWV) at the PSUM copy
SVV = 2         # V's effective fp8 scale (max |4v| ~ 19 < 240)
SWO = 7
SO = 2          # otn scaled by 2^2
CEXP = 2.5      # exp(s - CEXP); max causal score ~7.36 -> max p ~130 < 240

_CACHE = {}

TRACE = False
LAST_EXEC_NS = None


def _pair2(ap):
    """[128, 2*n] -> [128, 2, n] DoubleRow view."""
    n = ap.shape[-1] // 2
    return ap.rearrange("p (two n) -> p two n", two=2, n=n)


def _build_nc():
    nc = bacc.Bacc("TRN2", target_bir_lowering=False, debug=False,
                   num_devices=NCORES)

    xw_d = nc.declare_dram_parameter("xw", [SC, DP, 128, 3072], E4,
                                     isOutput=False)
    wq_d = nc.declare_dram_parameter("wq2", [DP, 128, 2048], E4,
                                     isOutput=False)
    wo_d = nc.declare_dram_parameter("wo4", [8, 128, 4096], E4,
                                     isOutput=False)
    cs_d = nc.declare_dram_parameter("cs4", [SC, HD, 2048], BF16,
                                     isOutput=False)
    perm_d = nc.declare_dram_parameter("perm", [HD, HD], BF16, isOutput=False)
    masks_d = nc.declare_dram_parameter("masks", [128, 4, 512], BF16,
                                        isOutput=False)
    onr_d = nc.declare_dram_parameter("ones_red", [128, 256], E4,
                                      isOutput=False)
    onb_d = nc.declare_dram_parameter("ones_bc", [1, 128], F32R,
                                      isOutput=False)
    ident_d = nc.declare_dram_parameter("ident", [HD, HD], BF16,
                                        isOutput=False)
    out_d = nc.declare_dram_parameter("out", [8, 8, 128, 1024], BF16,
                                      isOutput=True)

    with tile.TileContext(nc) as tc:
        from contextlib import ExitStack
        ctx = ExitStack()
        with ctx:
            wpool = ctx.enter_context(tc.tile_pool(name="wpool", bufs=16))
            xpool = ctx.enter_context(tc.tile_pool(name="xpool", bufs=3))
            qtp = ctx.enter_context(tc.tile_pool(name="qtp", bufs=9))
            otnp = ctx.enter_context(tc.tile_pool(name="otnp", bufs=16))
            tabp = ctx.enter_context(tc.tile_pool(name="tabp", bufs=2))
            ktp = ctx.enter_context(tc.tile_pool(name="ktp", bufs=4))
            vnp = ctx.enter_context(tc.tile_pool(name="vnp", bufs=8))
            stg = ctx.enter_context(tc.tile_pool(name="stg", bufs=4))
            rawp = ctx.enter_context(tc.tile_pool(name="rawp", bufs=6))
            ptp = ctx.enter_context(tc.tile_pool(name="ptp", bufs=40))
            mkp = ctx.enter_context(tc.tile_pool(name="mkp", bufs=1))
            cst = ctx.enter_context(tc.tile_pool(name="cst", bufs=1))
            rcp = ctx.enter_context(tc.tile_pool(name="rcp", bufs=4))
            ostp = ctx.enter_context(tc.tile_pool(name="ostp", bufs=3))
            psA = ctx.enter_context(
                tc.tile_pool(name="psA", bufs=6, space=bass.MemorySpace.PSUM))
            psB = ctx.enter_context(
                tc.tile_pool(name="psB", bufs=2, space=bass.MemorySpace.PSUM))

            perm_t = cst.tile([HD, HD], BF16, name="perm_t")
            ident_t = cst.tile([HD, HD], BF16, name="ident_t")
            onr_t = cst.tile([128, 256], E4, name="onr_t")
            onb_t = cst.tile([1, 128], F32R, name="onb_t")
            nexp_t = cst.tile([128, 1], F32, name="nexp_t")
            mask_t = mkp.tile([128, 4, 512], BF16, name="mask4")

            def table_loads():
                yield lambda: nc.sync.dma_start(perm_t[:], perm_d[:])
                yield lambda: nc.scalar.dma_start(ident_t[:], ident_d[:])
                yield lambda: nc.sync.dma_start(onr_t[:], onr_d[:])
                yield lambda: nc.scalar.dma_start(onb_t[:], onb_d[:])
                yield lambda: nc.sync.dma_start(mask_t[:], masks_d[:])
                yield lambda: nc.gpsimd.memset(nexp_t[:], -CEXP)

            wq_tiles = [None] * DP

            # persistent activations (bf16 for q/k, fp8 hi/lo pairs for V)
            QTc = [[qtp.tile([HD, 512], BF16, tag="qtc", name=f"qt{h}_{c}")
                    for c in range(SC)] for h in range(QH)]
            KTc = [ktp.tile([HD, 512], BF16, tag="ktc", name=f"kt{c}")
                   for c in range(SC)]
            Vhc = [vnp.tile([128, 4, 128], E4, tag="vnc", name=f"vh{c}")
                   for c in range(SC)]
            Vlc = [vnp.tile([128, 4, 128], E4, tag="vnc", name=f"vl{c}")
                   for c in range(SC)]

            def rope_copy(acc_ps, eng):
                raw = rawp.tile([128, 512], BF16, tag="raw", name="rope_raw")
                if eng == "act":
                    nc.scalar.activation(raw[:], acc_ps[:], AF.Copy)
                else:
                    nc.vector.tensor_copy(raw[:], acc_ps[:])
                return raw

            def rope_combine(raw, dest, cc, sn):
                rot = psB.tile([128, 512], F32, tag="tmp", name="rope_rot")
                nc.tensor.matmul(rot[:], perm_t[:], raw[:], start=True,
                                 stop=True)
                t1 = stg.tile([128, 512], BF16, tag="stg", name="rope_t1")
                nc.vector.tensor_mul(t1[:], raw[:], cc)
                t2 = stg.tile([128, 512], BF16, tag="stg", name="rope_t2")
                nc.vector.tensor_mul(t2[:], rot[:], sn)
                nc.vector.tensor_add(dest[:], t1[:], t2[:])

            boundary_pe = [None]

            def emit_boundary_pe():
                if boundary_pe[0] is not None:
                    boundary_pe[0]()
                    boundary_pe[0] = None

            cs_tiles = [None] * SC

            # otn hi/lo head-pair tiles: [c][hp] slot h%2
            otn_hi = [[None, None] for _ in range(SC)]
            otn_lo = [[None, None] for _ in range(SC)]
            tails = []

            def make_tail(c, h, ot, dsum):
                def tail():
                    dsg = rcp.tile([1, 512], F32R, tag="rc", name="dsg")
                    rc = rcp.tile([1, 512], F32R, tag="rc", name="rc")
                    with nc.allow_low_precision(reason="fp22 softmax recip"):
                        nc.vector.tensor_scalar_max(dsg[:], dsum[0:1, :],
                                                    1e-30)
                        nc.vector.reciprocal(rc[:], dsg[:])
                    bc = psA.tile([128, 512], F32, tag="acc", name="bc")
                    nc.tensor.matmul(bc[:], onb_t[:], rc[:], start=True,
                                     stop=True)
                    bcs = stg.tile([128, 512], BF16, tag="stg", name="bcs")
                    nc.scalar.activation(bcs[:], bc[:], AF.Copy)
                    ob = stg.tile([128, 512], BF16, tag="stg", name="otn_bf")
                    nc.vector.tensor_mul(ob[:], ot[:], bcs[:])
                    hi = otn_hi[c][h // 2][:, h % 2, :]
                    nc.vector.tensor_copy(hi, ob[:])
                    nc.vector.tensor_sub(otn_lo[c][h // 2][:, h % 2, :],
                                         ob[:], hi)
                return tail

            def score_pair(c, h, pr, pool):
                """Score + exp for j-tile pair pr of head h, chunk c."""
                qch = QTc[h][c][:]
                pp = ptp.tile([128, 2, 512], E4, tag="pt", name="pt")
                for jj in range(2):
                    jt = 2 * pr + jj
                    stp = pool.tile([128, 512], F32,
                                    tag="acc" if pool is psA else "tmp",
                                    name="stp")
                    nc.tensor.matmul(
                        stp[:],
                        KTc[jt // 4][:, 128 * (jt % 4):128 * (jt % 4 + 1)],
                        qch, start=True, stop=True)
                    pslot = pp[:, jj, :]
                    m = jt - 4 * c
                    if m >= 0:
                        # exp can exceed fp8 range above the diagonal;
                        # stage in bf16 so inf*0 never reaches pt
                        pe = stg.tile([128, 512], BF16, tag="stg",
                                      name="pe_t")
                        nc.scalar.activation(pe[:], stp[:], AF.Exp,
                                             bias=nexp_t[:])
                        nc.vector.tensor_mul(pslot, pe[:], mask_t[:, m, :])
                    else:
                        nc.scalar.activation(pslot, stp[:], AF.Exp,
                                             bias=nexp_t[:])
                return pp

            # pre[(h, pr)] -> pt pair tile, for the chunk whose scores were
            # interleaved into the following QKV dp-loop
            pre_pt = {}

            def attn_chunk(c):
                for hp in range(2):
                    otn_hi[c][hp] = otnp.tile([128, 2, 512], E4, tag="otn",
                                              name=f"oh{c}_{hp}")
                    otn_lo[c][hp] = otnp.tile([128, 2, 512], E4, tag="otn",
                                              name=f"ol{c}_{hp}")
                npair = 2 * c + 2
                for h in range(QH):
                    ot = psA.tile([128, 512], F32, tag="acc", name="ot_ps")
                    dsum = psB.tile([128, 512], F32, tag="tmp", name="dsum")
                    pairs = [None] * npair

                    def accum(pr, pp, ot=ot, dsum=dsum, npair=npair):
                        st = pr == 0
                        sp = pr == npair - 1
                        vh = Vhc[pr // 2][:, 2 * (pr % 2):2 * (pr % 2) + 2, :]
                        vl = Vlc[pr // 2][:, 2 * (pr % 2):2 * (pr % 2) + 2, :]
                        nc.tensor.matmul(ot[:], vh, pp[:], start=st,
                                         stop=False, perf_mode=DR)
                        nc.tensor.matmul(ot[:], vl, pp[:], start=False,
                                         stop=sp, perf_mode=DR)
                        nc.tensor.matmul(dsum[:], _pair2(onr_t[:]), pp[:],
                                         start=st, stop=sp, perf_mode=DR)

                    done = 0
                    for pr in range(npair):
                        pairs[pr] = pre_pt.pop((h, pr), None)
                        if pairs[pr] is None:
                            # in-block score: defer AV two pairs for exp
                            pairs[pr] = score_pair(c, h, pr, psA)
                            while done < pr - 1:
                                accum(done, pairs[done])
                                done += 1
                        else:
                            # precomputed pt is ready; accumulate right away
                            while done <= pr:
                                accum(done, pairs[done])
                                done += 1
                    while done < npair:
                        accum(done, pairs[done])
                        done += 1
                    tails.append(make_tail(c, h, ot, dsum))
                    if len(tails) > 1:
                        tails.pop(0)()
                while tails:
                    tails.pop(0)()

            emit_attn = [attn_chunk]

            for sc in range(SC):
                qps = [psA.tile([128, 512], F32, tag="acc", name=f"qps{h}")
                       for h in range(QH)]
                kps = psA.tile([128, 512], F32, tag="acc", name="kps")
                vps = psA.tile([128, 512], F32, tag="acc", name="vps")
                # scores+exps of the previous chunk's attention are emitted
                # inside this dp loop (via the psB banks) so ACT works
                # through the exps while PE runs QKV
                pend = []
                if sc >= 1:
                    pend = [(h, pr) for h in range(QH)
                            for pr in range(2 * (sc - 1) + 2)]
                for dp in range(DP):
                    if sc == 0:
                        wt = wpool.tile([128, 2048], E4, tag="w",
                                        name=f"wq{dp}")
                        nc.scalar.dma_start(wt[:], wq_d[dp])
                        wq_tiles[dp] = wt
                    xt = xpool.tile([128, 3072], E4, tag="x", name="xt")
                    nc.sync.dma_start(xt[:], xw_d[sc, dp])
                    if sc == 0:
                        if dp == 0:
                            _tl = table_loads()
                        next(_tl, lambda: None)()
                    if dp == 1:
                        emit_boundary_pe()
                    if dp == 8:
                        cc4 = tabp.tile([128, 2048], BF16, tag="tab",
                                        name="cc4")
                        nc.sync.dma_start(cc4[:], cs_d[sc])
                        cs_tiles[sc] = cc4
                    xh = _pair2(xt[:, 0:1024])
                    xl = _pair2(xt[:, 1024:2048])
                    wkh = _pair2(xt[:, 2048:2304])
                    wkl = _pair2(xt[:, 2304:2560])
                    wvh = _pair2(xt[:, 2560:2816])
                    wvl = _pair2(xt[:, 2816:3072])
                    wqh = _pair2(wq_tiles[dp][:, 0:1024])
                    wql = _pair2(wq_tiles[dp][:, 1024:2048])
                    st = dp == 0
                    sp = dp == DP - 1
                    for h in range(QH):
                        hsl = slice(HD * h, HD * (h + 1))
                        nc.tensor.matmul(qps[h][:], wqh[:, :, hsl], xh[:],
                                         start=st, stop=False, perf_mode=DR)
                        nc.tensor.matmul(qps[h][:], wqh[:, :, hsl], xl[:],
                                         start=False, stop=False,
                                         perf_mode=DR)
                        nc.tensor.matmul(qps[h][:], wql[:, :, hsl], xh[:],
                                         start=False, stop=sp, perf_mode=DR)
                    nc.tensor.matmul(kps[:], wkh, xh[:], start=st,
                                     stop=False, perf_mode=DR)
                    nc.tensor.matmul(kps[:], wkh, xl[:], start=False,
                                     stop=False, perf_mode=DR)
                    nc.tensor.matmul(kps[:], wkl, xh[:], start=False,
                                     stop=sp, perf_mode=DR)
                    nc.tensor.matmul(vps[:], wvh, xh[:], start=st,
                                     stop=False, perf_mode=DR)
                    nc.tensor.matmul(vps[:], wvh, xl[:], start=False,
                                     stop=False, perf_mode=DR)
                    nc.tensor.matmul(vps[:], wvl, xh[:], start=False,
                                     stop=sp, perf_mode=DR)
                    if dp >= 2:
                        for _ in range(2):
                            if pend:
                                h, pr = pend.pop(0)
                                pre_pt[(h, pr)] = score_pair(sc - 1, h, pr,
                                                             psB)

                raw_k = rope_copy(kps, "act")
                vt_sb = rawp.tile([128, 512], BF16, tag="raw", name="vt_sb")
                nc.vector.tensor_scalar_mul(vt_sb[:], vps[:],
                                            2.0 ** (SVV - SWV))
                raw_q = [None] * QH
                raw_q[0] = rope_copy(qps[0], "dve")

                def boundary(sc=sc, raw_k=raw_k, vt_sb=vt_sb, raw_q=raw_q):
                    cc4 = cs_tiles[sc]
                    cq = cc4[:, 0:512]
                    sq = cc4[:, 512:1024]
                    ck = cc4[:, 1024:1536]
                    sk = cc4[:, 1536:2048]
                    rope_combine(raw_k, KTc[sc], ck, sk)
                    rope_combine(raw_q[0], QTc[0][sc], cq, sq)
                    for k4 in range(4):
                        vtp = psB.tile([128, 128], BF16, tag="tmp",
                                       name="vtp")
                        nc.tensor.transpose(
                            vtp[:], vt_sb[:, 128 * k4:128 * (k4 + 1)],
                            ident_t[:])
                        nc.scalar.activation(Vhc[sc][:, k4, :], vtp[:],
                                             AF.Copy)
                        nc.vector.tensor_sub(Vlc[sc][:, k4, :], vtp[:],
                                             Vhc[sc][:, k4, :])
                    for h in range(1, QH):
                        rope_combine(raw_q[h], QTc[h][sc], cq, sq)

                boundary_pe[0] = boundary
                if sc >= 1:
                    emit_attn[0](sc - 1)
                for h in range(1, QH):
                    raw_q[h] = rope_copy(qps[h],
                                         "act" if h % 2 == 0 else "dve")
                if sc == SC - 1:
                    wo_tiles = []
                    for dc in range(8):
                        wo = wpool.tile([128, 4096], E4, tag="w",
                                        name=f"wo{dc}")
                        nc.sync.dma_start(wo[:], wo_d[dc])
                        wo_tiles.append(wo)

                    emit_boundary_pe()

            # ---- phase 3: output projection (row-parallel partial), with
            # the last attention chunk's scores/exps interleaved so its exp
            # cost hides under o_proj's PE work
            def o_block(dc, c):
                wo = wo_tiles[dc]
                woh = [_pair2(wo[:, 0:1024]), _pair2(wo[:, 1024:2048])]
                wol = [_pair2(wo[:, 2048:3072]), _pair2(wo[:, 3072:4096])]
                for lp in range(2):
                    ost = ostp.tile([128, 1024], BF16, tag="ost",
                                    name="ost")
                    for k2 in range(2):
                        kk = 2 * lp + k2
                        ksl = slice(128 * kk, 128 * (kk + 1))
                        acc = psA.tile([128, 512], F32, tag="acc",
                                       name="oacc")
                        for hp in range(2):
                            oh = otn_hi[c][hp][:, :, ksl]
                            ol = otn_lo[c][hp][:, :, ksl]
                            nc.tensor.matmul(acc[:], oh, woh[hp][:],
                                             start=(hp == 0), stop=False,
                                             perf_mode=DR)
                            nc.tensor.matmul(acc[:], ol, woh[hp][:],
                                             start=False, stop=False,
                                             perf_mode=DR)
                            nc.tensor.matmul(acc[:], oh, wol[hp][:],
                                             start=False, stop=(hp == 1),
                                             perf_mode=DR)
                        dstc = ost[:, 512 * k2:512 * (k2 + 1)]
                        if k2 == 0:
                            nc.vector.tensor_copy(dstc, acc[:])
                        else:
                            nc.scalar.activation(dstc, acc[:], AF.Copy)
                    eng = nc.sync if lp == 0 else nc.scalar
                    eng.dma_start(out_d[dc, 2 * c + lp], ost[:])

            def o_proj_all():
                pend3 = [(h, pr) for h in range(QH)
                         for pr in range(2 * (SC - 1) + 2)]
                for c in range(SC - 1):
                    for dc in range(8):
                        for _ in range(2):
                            if pend3:
                                h, pr = pend3.pop(0)
                                pre_pt[(h, pr)] = score_pair(SC - 1, h, pr,
                                                             psB)
                        o_block(dc, c)
                emit_attn[0](SC - 1)
                for dc in range(8):
                    o_block(dc, SC - 1)

            o_proj_all()

    nc.compile()
    return nc


def _q8(x):
    return x.astype(NE4)


def _split8(x):
    h = x.astype(NE4)
    l = (x - h.astype(np.float32)).astype(NE4)
    return h, l


def _host_tables():
    pos = np.arange(S, dtype=np.float64)
    inv_freq = ROPE_BASE ** (-np.arange(0, HD, 2, dtype=np.float64) / HD)
    ang = np.outer(pos, inv_freq)  # [S, HD/2]
    cos = np.cos(ang).T.astype(np.float32)  # [HD/2, S]
    sin = np.sin(ang).T.astype(np.float32)
    cos2 = np.repeat(cos, 2, axis=0)  # [HD, S]
    sin2 = np.repeat(sin, 2, axis=0)
    sin2[0::2, :] *= -1.0  # even rows get -sin, odd rows +sin

    perm = np.zeros((HD, HD), dtype=np.float32)
    for i in range(HD):
        perm[i ^ 1, i] = 1.0

    masks = np.zeros((128, 4, 512), dtype=np.float32)
    jr = np.arange(128)[:, None]
    ir = np.arange(512)[None, :]
    for m in range(4):
        masks[:, m, :] = np.where(jr + 128 * m <= ir, 1.0, 0.0)

    return cos2, sin2, perm, masks


def kernel(x, Wq, Wk, Wv, Wo):
    global LAST_EXEC_NS
    if "nc" not in _CACHE:
        _CACHE["nc"] = _build_nc()
    nc = _CACHE["nc"]

    x = np.asarray(x, dtype=np.float32).reshape(S, D)
    Wq = np.asarray(Wq, dtype=np.float32)
    Wk = np.asarray(Wk, dtype=np.float32)
    Wv = np.asarray(Wv, dtype=np.float32)
    Wo = np.asarray(Wo, dtype=np.float32)

    xT = np.ascontiguousarray(x.T)  # [D, S]
    xh8, xl8 = _split8(xT)
    # [DP, 128, 2, 512] per (pair, row, slot, s-chunk col) built per chunk
    xh8r = xh8.reshape(DP, 2, 128, SC, 512)
    xl8r = xl8.reshape(DP, 2, 128, SC, 512)
    cos2, sin2, perm, masks = _host_tables()
    scale = np.float32(1.0 / np.sqrt(HD))

    # cs4[sc] = [cq | sq | ck | sk], each [HD, 512], bf16
    cs4 = np.empty((SC, HD, 2048), dtype=NBF)
    for scc in range(SC):
        cs = slice(512 * scc, 512 * (scc + 1))
        cs4[scc, :, 0:512] = (cos2[:, cs] * 2.0 ** -SWQ).astype(NBF)
        cs4[scc, :, 512:1024] = (sin2[:, cs] * 2.0 ** -SWQ).astype(NBF)
        cs4[scc, :, 1024:1536] = (cos2[:, cs] * 2.0 ** -SWK).astype(NBF)
        cs4[scc, :, 1536:2048] = (sin2[:, cs] * 2.0 ** -SWK).astype(NBF)

    ident = np.eye(HD, dtype=NBF)
    ones_red = np.zeros((128, 256), dtype=NE4)
    ones_red[:, 0] = 2.0 ** (SVV - SO)
    ones_red[:, 128] = 2.0 ** (SVV - SO)
    ones_bc = np.ones((1, 128), dtype=np.float32)

    in_maps = []
    for c in range(NCORES):
        qs = slice(QH * HD * c, QH * HD * (c + 1))
        ks = slice(HD * c, HD * (c + 1))
        wkh, wkl = _split8(Wk[:, ks] * 2.0 ** SWK)   # [D, 128] fp8
        wvh, wvl = _split8(Wv[:, ks] * 2.0 ** SWV)

        # xw[sc, dp] = [xh pair 1024 | xl pair 1024 | wkh 256 | wkl | wvh | wvl]
        xw = np.empty((SC, DP, 128, 3072), dtype=NE4)
        for scc in range(SC):
            xw[scc, :, :, 0:1024] = (
                xh8r[:, :, :, scc].transpose(0, 2, 1, 3).reshape(DP, 128, 1024))
            xw[scc, :, :, 1024:2048] = (
                xl8r[:, :, :, scc].transpose(0, 2, 1, 3).reshape(DP, 128, 1024))
        wk4 = np.stack([wkh, wkl, wvh, wvl], axis=0)  # [4, D, 128]
        wk4r = (wk4.reshape(4, DP, 2, 128, 128).transpose(1, 3, 0, 2, 4)
                .reshape(DP, 128, 1024))
        xw[:, :, :, 2048:3072] = wk4r[None]

        wqc = Wq[:, qs].astype(np.float32) * scale * 2.0 ** SWQ
        wqhh, wqll = _split8(wqc)   # [D, 512]
        wq2 = np.empty((DP, 128, 2048), dtype=NE4)
        wq2[:, :, 0:1024] = (wqhh.reshape(DP, 2, 128, 512)
                             .transpose(0, 2, 1, 3).reshape(DP, 128, 1024))
        wq2[:, :, 1024:2048] = (wqll.reshape(DP, 2, 128, 512)
                                .transpose(0, 2, 1, 3).reshape(DP, 128, 1024))

        # wo4[dc] = [Woh hp0 | Woh hp1 | Wol hp0 | Wol hp1], each [128,2,512]
        woc = Wo[qs, :].astype(np.float32) * 2.0 ** SWO  # [512, D]
        woh, wol = _split8(woc)
        wo4 = np.empty((8, 128, 4096), dtype=NE4)
        for part, w8 in ((0, woh), (1, wol)):
            # w8 [512, 4096] -> [4 heads, 128, 8 dc, 512]
            wr = w8.reshape(4, 128, 8, 512)
            for dc in range(8):
                for hp in range(2):
                    blk = wr[2 * hp:2 * hp + 2, :, dc, :]  # [2, 128, 512]
                    off = 2048 * part + 1024 * hp
                    wo4[dc, :, off:off + 1024] = (
                        blk.transpose(1, 0, 2).reshape(128, 1024))

        in_maps.append({
            "xw": xw,
            "wq2": wq2,
            "wo4": wo4,
            "cs4": cs4,
            "perm": perm.astype(NBF),
            "masks": masks.astype(NBF),
            "ones_red": ones_red,
            "ones_bc": ones_bc,
            "ident": ident,
        })

    res = run_bass_kernel_spmd(nc, in_maps, list(range(NCORES)),
                               trace=TRACE)
    LAST_EXEC_NS = res.exec_time_ns

    acc = res.results[0]["out"].astype(np.float32)
    for c in range(1, NCORES):
        acc = acc + res.results[c]["out"].astype(np.float32)
    acc *= 2.0 ** -(SO + SWO)
    # out[dc, sp2, p, k2*512 + col] -> out[(2*sp2+k2)*128 + p, dc*512 + col]
    out = (acc.reshape(8, 8, 128, 2, 512).transpose(1, 3, 2, 0, 4)
           .reshape(S, D))
    return np.ascontiguousarray(out).reshape(1, S, D)
